# revision 1
# baseline (speedup 1.0000x reference)
"""Multi-head self-attention (RoPE, causal) Trainium2 Bass kernel.

Full inputs in, full output out. Sharding: 8 cores = 2 batch x 4 head-groups
(4 heads each). Per core: qkv projection (fp32r matmuls), RoPE on DVE,
streaming causal attention (S^T orientation: softmax reduction along
partitions via a ones-column in V-hat), output projection partial. Host sums
the 4 per-batch partials and adds the (bv @ Wproj + bproj) constant.

Self-contained: hardcodes all shapes for B=2, T=2048, D=1024, H=16, hd=64.
"""
from contextlib import ExitStack

import numpy as np

from concourse import bacc, mybir, tile
from concourse.bass_utils import run_bass_kernel_spmd

f32 = mybir.dt.float32
f32r = mybir.dt.float32r
EXP = mybir.ActivationFunctionType.Exp
IDENT = mybir.ActivationFunctionType.Identity

B, T, D = 2, 2048, 1024
H, HD = 16, 64
HALF = HD // 2  # 32
HPC = 4  # heads per core
BASE = 10000.0
NTQ = 4  # token quarters of 512 (qkv phase)
NQC = 4  # query chunks of 512 (attention phase)
NKT = 16  # key tiles of 128
VW = HPC * (HD + 1)  # 260: v-hat columns per token tile


def _build():
    nc = bacc.Bacc("TRN2", target_bir_lowering=False, debug=False, num_devices=8)

    xT = nc.dram_tensor("xT", [D, T], f32r, kind="ExternalInput").ap()
    wqk = nc.dram_tensor("wqk", [D, 512], f32r, kind="ExternalInput").ap()
    wv = nc.dram_tensor("wv", [D, 256], f32r, kind="ExternalInput").ap()
    wp = nc.dram_tensor("wp", [256, D], f32r, kind="ExternalInput").ap()
    bqk = nc.dram_tensor("bqk", [128, 4], f32, kind="ExternalInput").ap()
    cos4 = nc.dram_tensor("cos4", [128, T], f32, kind="ExternalInput").ap()
    sin4 = nc.dram_tensor("sin4", [128, T], f32, kind="ExternalInput").ap()
    trimask = nc.dram_tensor("trimask", [128, 128], f32r, kind="ExternalInput").ap()
    ones64_d = nc.dram_tensor("ones64_d", [1, 64], f32r, kind="ExternalInput").ap()
    ones_pat = nc.dram_tensor("ones_pat", [128, 64], f32r, kind="ExternalInput").ap()
    outT = nc.dram_tensor("outT", [D, T], f32, kind="ExternalOutput").ap()

    with tile.TileContext(nc) as tc, ExitStack() as ctx:
        consts = ctx.enter_context(tc.tile_pool(name="consts", bufs=1))
        wpool = ctx.enter_context(tc.tile_pool(name="wpool", bufs=1))
        xt_pool = ctx.enter_context(tc.tile_pool(name="xt", bufs=6))
        qkstage = ctx.enter_context(tc.tile_pool(name="qkstage", bufs=8))
        tmp_pool = ctx.enter_context(tc.tile_pool(name="tmp", bufs=2))
        vh_pool = ctx.enter_context(tc.tile_pool(name="vh", bufs=1))
        at_pool = ctx.enter_context(tc.tile_pool(name="at", bufs=5))
        small = ctx.enter_context(tc.tile_pool(name="small", bufs=2))
        rb_pool = ctx.enter_context(tc.tile_pool(name="rb", bufs=2))
        ot_pool = ctx.enter_context(tc.tile_pool(name="ot", bufs=2))
        po_pool = ctx.enter_context(tc.tile_pool(name="po", bufs=3))

        # ---- weights on the sync queue interleaved with first x chunks;
        # ---- everything not needed immediately on the scalar HWDGE queue.
        wqk_t = wpool.tile([128, 8, 512], f32r, tag="wqk_t")
        wv_t = wpool.tile([128, 8, 256], f32r, tag="wv_t")
        wp_t = wpool.tile([128, 2, D], f32r, tag="wp_t")
        cos_t = consts.tile([128, T], f32, tag="cos_t")
        sin_t = consts.tile([128, T], f32, tag="sin_t")
        tri_t = consts.tile([128, 128], f32r, tag="tri_t")
        bqk_t = consts.tile([128, 4], f32, tag="bqk_t")
        ones64 = consts.tile([1, 64], f32r, tag="ones64")

        for dn in range(8):
            nc.scalar.dma_start(wv_t[:, dn, :], wv[dn * 128:(dn + 1) * 128, :])
        nc.scalar.dma_start(bqk_t[:], bqk)
        nc.scalar.dma_start(cos_t[:], cos4)
        nc.scalar.dma_start(sin_t[:], sin4)
        nc.scalar.dma_start(tri_t[:], trimask)
        nc.scalar.dma_start(ones64[:], ones64_d)
        for hd in range(2):
            nc.scalar.dma_start(wp_t[:, hd, :], wp[hd * 128:(hd + 1) * 128, :])

        # v-hat: [128, 16 tok-tiles x (4 heads x 65)]; col 64 of each head = 1.0
        vhat = vh_pool.tile([128, NKT * VW], f32r, tag="vhat")
        vh_ones = vhat[:, :].rearrange("p (t h c) -> p t h c", t=NKT,
                                       h=HPC)[:, :, :, HD:HD + 1]
        nc.scalar.dma_start(vh_ones,
                            ones_pat.rearrange("p (t h) -> p t h", t=NKT)[:, :, :, None])

        # qkv^T output stage tensors [128, T] each
        qE = qkstage.tile([128, T], f32, tag="qks")
        qO = qkstage.tile([128, T], f32, tag="qks")
        kE = qkstage.tile([128, T], f32, tag="qks")
        kO = qkstage.tile([128, T], f32, tag="qks")
        chunks = [qE, qO, kE, kO]

        # ---- phase 1: qkv matmuls ----
        with tc.tile_pool(name="ps_qk", bufs=4, space="PSUM") as ps_qk, \
             tc.tile_pool(name="ps_v", bufs=4, space="PSUM") as ps_v:
            for tq in range(NTQ):
                t0 = tq * 512
                xc = []
                for dn in range(8):
                    xt = xt_pool.tile([128, 512], f32r, tag="xt")
                    if tq == 0:
                        # interleave weight + activation loads so the first
                        # matmul can start after two small DMAs
                        nc.sync.dma_start(wqk_t[:, dn, :],
                                          wqk[dn * 128:(dn + 1) * 128, :])
                    nc.sync.dma_start(xt[:], xT[dn * 128:(dn + 1) * 128,
                                                t0:t0 + 512])
                    xc.append(xt)
                pqk = [ps_qk.tile([128, 512], f32, tag="ps_qk",
                                  name=f"pqk{tq}_{i}") for i in range(4)]
                pv = [ps_v.tile([128, 256], f32, tag="ps_v",
                                name=f"pv{tq}_{i}") for i in range(4)]
                for dn in range(8):
                    for ch in range(4):
                        nc.tensor.matmul(
                            pqk[ch][:],
                            wqk_t[:, dn, ch * 128:(ch + 1) * 128],
                            xc[dn][:],
                            start=(dn == 0), stop=(dn == 7))
                    for tt in range(4):  # token tiles of 128 within quarter
                        nc.tensor.matmul(
                            pv[tt][:],
                            xc[dn][:, tt * 128:(tt + 1) * 128],
                            wv_t[:, dn, :],
                            start=(dn == 0), stop=(dn == 7))
                # psum -> sbuf copies (+ bias for q,k on ACT)
                for ch in range(4):
                    nc.scalar.activation(
                        chunks[ch][:, t0:t0 + 512], pqk[ch][:], IDENT,
                        bias=bqk_t[:, ch:ch + 1], scale=1.0)
                for tt in range(4):
                    tglob = tq * 4 + tt
                    dst = vhat[:, tglob * VW:(tglob + 1) * VW].rearrange(
                        "p (h c) -> p h c", h=HPC)[:, :, 0:HD]
                    nc.vector.tensor_copy(
                        dst, pv[tt][:].rearrange("p (h c) -> p h c", h=HPC))

        # ---- phase 2: rope + permute ----
        qF = qkstage.tile([128, T], f32r, tag="qks")
        qS = qkstage.tile([128, T], f32r, tag="qks")
        kF = qkstage.tile([128, T], f32r, tag="qks")
        kS = qkstage.tile([128, T], f32r, tag="qks")
        for (E, O, F, S) in ((qE, qO, qF, qS), (kE, kO, kF, kS)):
            for tq in range(NTQ):
                sl = slice(tq * 512, tq * 512 + 512)
                t1 = tmp_pool.tile([128, 512], f32, tag="tmp")
                t2 = tmp_pool.tile([128, 512], f32, tag="tmp")
                nc.vector.tensor_mul(t1[:], E[:, sl], cos_t[:, sl])
                nc.vector.tensor_mul(t2[:], O[:, sl], sin_t[:, sl])
                nc.vector.tensor_sub(F[:, sl], t1[:], t2[:])
                t3 = tmp_pool.tile([128, 512], f32, tag="tmp")
                t4 = tmp_pool.tile([128, 512], f32, tag="tmp")
                nc.vector.tensor_mul(t3[:], E[:, sl], sin_t[:, sl])
                nc.vector.tensor_mul(t4[:], O[:, sl], cos_t[:, sl])
                nc.vector.tensor_add(S[:, sl], t3[:], t4[:])

        # permute (h-major 32-blocks F/S) -> per-head-64-contiguous qr/kr
        qrA = qkstage.tile([128, T], f32r, tag="qks")
        qrB = qkstage.tile([128, T], f32r, tag="qks")
        krA = qkstage.tile([128, T], f32r, tag="qks")
        krB = qkstage.tile([128, T], f32r, tag="qks")
        for (F, S, rA, rB) in ((qF, qS, qrA, qrB), (kF, kS, krA, krB)):
            for h in range(4):
                dst = rA if h < 2 else rB
                r0 = (h % 2) * 64
                nc.sync.dma_start(dst[r0:r0 + 32, :], F[h * 32:(h + 1) * 32, :])
                nc.sync.dma_start(dst[r0 + 32:r0 + 64, :],
                                  S[h * 32:(h + 1) * 32, :])

        # ---- phase 3: attention (head-outer for stationary reuse) ----
        otA = ot_pool.tile([128, T], f32r, tag="ot")
        otB = ot_pool.tile([128, T], f32r, tag="ot")
        with tc.tile_pool(name="ps_s", bufs=4, space="PSUM") as ps_s, \
             tc.tile_pool(name="ps_o", bufs=4, space="PSUM") as ps_o:
            for h in range(HPC):
                qr = qrA if h < 2 else qrB
                kr = krA if h < 2 else krB
                r0 = (h % 2) * 64
                po = [ps_o.tile([65, 512], f32, tag="ps_o",
                                name=f"po{h}_{i}") for i in range(NQC)]
                for kt in range(NKT):
                    k0 = kt * 128
                    qc_lo = kt // 4  # first query chunk attending this k-tile
                    ats = {}
                    for qc in range(qc_lo, NQC):
                        q0 = qc * 512
                        col_lo = k0 - q0 if k0 > q0 else 0  # diag sub-range
                        pss = ps_s.tile([128, 512], f32, tag="ps_s",
                                        name=f"pss{h}_{kt}_{qc}")
                        nc.tensor.matmul(
                            pss[:, col_lo:512],
                            kr[r0:r0 + 64, k0:k0 + 128],
                            qr[r0:r0 + 64, q0 + col_lo:q0 + 512],
                            start=True, stop=True)
                        at = at_pool.tile([128, 512], f32r, tag="at",
                                          name=f"at{h}_{kt}_{qc}")
                        nc.scalar.activation(at[:, col_lo:512],
                                             pss[:, col_lo:512], EXP)
                        if qc == qc_lo and k0 >= q0:
                            nc.vector.tensor_mul(
                                at[:, col_lo:col_lo + 128],
                                at[:, col_lo:col_lo + 128], tri_t[:])
                        ats[qc] = (at, col_lo)
                    for qc in range(qc_lo, NQC):
                        at, col_lo = ats[qc]
                        nc.tensor.matmul(
                            po[qc][:, col_lo:512],
                            vhat[:, kt * VW + h * 65:kt * VW + (h + 1) * 65],
                            at[:, col_lo:512],
                            start=(kt == 0), stop=(kt == 4 * qc + 3),
                            skip_group_check=True)
                # normalize: approx-recip of denom row, broadcast, multiply
                for qc in range(NQC):
                    q0 = qc * 512
                    recip = small.tile([1, 512], f32r, tag="recip",
                                       name=f"recip{h}_{qc}")
                    with nc.allow_low_precision(reason="f32r recip feeds PE broadcast"):
                        nc.vector.reciprocal(recip[:], po[qc][64:65, :])
                    prb = ps_s.tile([64, 512], f32, tag="ps_s",
                                    name=f"prb{h}_{qc}")
                    nc.tensor.matmul(prb[:], ones64[:], recip[:],
                                     start=True, stop=True)
                    rb = rb_pool.tile([64, 512], f32, tag="rb",
                                      name=f"rb{h}_{qc}")
                    nc.scalar.copy(rb[:], prb[:])
                    ot = otA if h < 2 else otB
                    nc.vector.tensor_mul(ot[r0:r0 + 64, q0:q0 + 512],
                                         po[qc][0:64, :], rb[:])

            # ---- phase 4: projection (stationary reuse across qc) ----
            for oc in range(8):
                pj = [ps_s.tile([128, 512], f32, tag="ps_s",
                                name=f"pj{oc}_{i}") for i in range(NQC)]
                for hd in range(2):
                    src = otA if hd == 0 else otB
                    for qc in range(NQC):
                        nc.tensor.matmul(
                            pj[qc][:], wp_t[:, hd, oc * 128:(oc + 1) * 128],
                            src[:, qc * 512:qc * 512 + 512],
                            start=(hd == 0), stop=(hd == 1))
                for qc in range(NQC):
                    ob = po_pool.tile([128, 512], f32, tag="po",
                                      name=f"ob{oc}_{qc}")
                    nc.any.tensor_copy(ob[:], pj[qc][:])
                    nc.sync.dma_start(
                        outT[oc * 128:(oc + 1) * 128,
                             qc * 512:qc * 512 + 512], ob[:])

    nc.compile()
    return nc


_NC = None


def _get_nc():
    global _NC
    if _NC is None:
        _NC = _build()
    return _NC


def _host_prep(x, Wqkv, bqkv, Wproj, bproj, pos):
    """Build the 8 per-core input maps."""
    x = np.asarray(x, dtype=np.float32)
    Wqkv = np.asarray(Wqkv, dtype=np.float32)
    bqkv = np.asarray(bqkv, dtype=np.float32)
    Wproj = np.asarray(Wproj, dtype=np.float32)
    bproj = np.asarray(bproj, dtype=np.float32)
    pos = int(np.asarray(pos))

    scale = HD ** -0.5
    # rope tables, layout [128 = 4 heads x 32 thetas (h-major), T]
    theta = 1.0 / BASE ** (np.arange(HALF, dtype=np.float32) / HALF)
    angles = np.outer(np.arange(pos, pos + T, dtype=np.float32), theta)  # [T,32]
    cosT = np.cos(angles).T.astype(np.float32)  # [32, T]
    sinT = np.sin(angles).T.astype(np.float32)
    cos4 = np.ascontiguousarray(np.tile(cosT, (4, 1)))  # [128, T]
    sin4 = np.ascontiguousarray(np.tile(sinT, (4, 1)))

    tri = np.tril(np.ones((128, 128), dtype=np.float32)).T  # m[p,j]=1 if p<=j
    tri = np.ascontiguousarray(tri)

    in_maps = []
    for c in range(8):
        b, hg = c // 4, c % 4
        heads = [4 * hg + h for h in range(HPC)]
        permE = np.array([h * HD + 2 * i for h in heads for i in range(HALF)])
        permO = permE + 1
        wqk_np = np.concatenate([
            Wqkv[:, permE] * scale,          # qE
            Wqkv[:, permO] * scale,          # qO
            Wqkv[:, D + permE],              # kE
            Wqkv[:, D + permO],              # kO
        ], axis=1)
        bqk_np = np.stack([
            bqkv[permE] * scale, bqkv[permO] * scale,
            bqkv[D + permE], bqkv[D + permO],
        ], axis=1)
        wv_np = Wqkv[:, 2 * D + 256 * hg: 2 * D + 256 * (hg + 1)]
        wp_np = Wproj[256 * hg: 256 * (hg + 1), :]
        in_maps.append({
            "xT": np.ascontiguousarray(x[b].T),
            "wqk": np.ascontiguousarray(wqk_np),
            "wv": np.ascontiguousarray(wv_np),
            "wp": np.ascontiguousarray(wp_np),
            "bqk": np.ascontiguousarray(bqk_np, dtype=np.float32),
            "cos4": cos4,
            "sin4": sin4,
            "trimask": tri,
            "ones64_d": np.ones((1, 64), dtype=np.float32),
            "ones_pat": np.ones((128, 64), dtype=np.float32),
        })
    const_vec = bqkv[2 * D:] @ Wproj + bproj  # exact host-side bias handling
    return in_maps, const_vec


def kernel(x, Wqkv, bqkv, Wproj, bproj, pos, **kw):
    in_maps, const_vec = _host_prep(x, Wqkv, bqkv, Wproj, bproj, pos)
    nc = _get_nc()
    res = run_bass_kernel_spmd(nc, in_maps, core_ids=list(range(8))).results
    out = np.empty((B, T, D), dtype=np.float32)
    for b in range(B):
        acc = res[4 * b]["outT"].copy()
        for c in range(4 * b + 1, 4 * b + 4):
            acc += res[c]["outT"]
        out[b] = acc.T + const_vec
    return out



# revision 21
# speedup vs baseline: 1.2560x; 1.2560x over previous
"""Multi-head self-attention (RoPE, causal) Trainium2 Bass kernel.

Full inputs in, full output out. Sharding: 8 cores = 2 batch x 4 head-groups
(4 heads each). Per core: qkv projection, RoPE on DVE, streaming causal
attention (S^T orientation: softmax reduction along partitions via a
ones-column in V-hat), output projection partial. Host sums the 4 per-batch
partials and adds the (bv @ Wproj + bproj) constant.

v2: all matmul operands in bf16 (PSUM accumulation stays fp32) — lower PE
power (avoids the sustained K=4/8 clock throttle seen with f32r), fast
weight loads, half the DMA bytes. Reciprocal via the fast custom-DVE
approximation. RoPE in bf16 for DVE 2x throughput. Output staged bf16.

Self-contained: hardcodes all shapes for B=2, T=2048, D=1024, H=16, hd=64.
"""
from contextlib import ExitStack

import numpy as np

from concourse import bacc, mybir, tile
from concourse.bass_utils import run_bass_kernel_spmd
from concourse.dve_ops import RECIP_APPROX_FAST_CONSTS, RECIPROCAL_APPROX_FAST

f32 = mybir.dt.float32
f32r = mybir.dt.float32r
bf16 = mybir.dt.bfloat16
EXP = mybir.ActivationFunctionType.Exp
IDENT = mybir.ActivationFunctionType.Identity

B, T, D = 2, 2048, 1024
H, HD = 16, 64
HALF = HD // 2  # 32
HPC = 4  # heads per core
BASE = 10000.0
NTQ = 4  # token quarters of 512 (qkv phase)
NQC = 4  # query chunks of 512 (attention phase)
NKT = 16  # key tiles of 128
VW = HPC * (HD + 1)  # 260: v-hat columns per token tile


def _build(dbg=False):
    nc = bacc.Bacc("TRN2", target_bir_lowering=False, debug=False, num_devices=8)

    xT = nc.dram_tensor("xT", [D, T], bf16, kind="ExternalInput").ap()
    wqk = nc.dram_tensor("wqk", [D, 512], bf16, kind="ExternalInput").ap()
    wv = nc.dram_tensor("wv", [D, 256], bf16, kind="ExternalInput").ap()
    wp = nc.dram_tensor("wp", [256, D], bf16, kind="ExternalInput").ap()
    bqk = nc.dram_tensor("bqk", [128, 4], f32, kind="ExternalInput").ap()
    cos4 = nc.dram_tensor("cos4", [128, T], bf16, kind="ExternalInput").ap()
    sin4 = nc.dram_tensor("sin4", [128, T], bf16, kind="ExternalInput").ap()
    trimask = nc.dram_tensor("trimask", [128, 128], bf16, kind="ExternalInput").ap()
    ones64_d = nc.dram_tensor("ones64_d", [1, 64], f32r, kind="ExternalInput").ap()
    sel4_d = nc.dram_tensor("sel4_d", [128, 256], f32r, kind="ExternalInput").ap()
    ones_pat = nc.dram_tensor("ones_pat", [128, 64], bf16, kind="ExternalInput").ap()
    outT = nc.dram_tensor("outT", [D, T], bf16, kind="ExternalOutput").ap()
    if dbg:
        dbg_qE = nc.dram_tensor("dbg_qE", [128, T], bf16, kind="ExternalOutput").ap()
        dbg_qF = nc.dram_tensor("dbg_qF", [128, T], bf16, kind="ExternalOutput").ap()
        dbg_qrA = nc.dram_tensor("dbg_qrA", [128, T], bf16, kind="ExternalOutput").ap()
        dbg_vhat = nc.dram_tensor("dbg_vhat", [128, NKT * VW], bf16, kind="ExternalOutput").ap()
        dbg_at = nc.dram_tensor("dbg_at", [128, 512], bf16, kind="ExternalOutput").ap()
        dbg_po = nc.dram_tensor("dbg_po", [65, 512], mybir.dt.float32, kind="ExternalOutput").ap()
        dbg_ot = nc.dram_tensor("dbg_ot", [128, T], bf16, kind="ExternalOutput").ap()

    with tile.TileContext(nc) as tc, ExitStack() as ctx:
        consts = ctx.enter_context(tc.tile_pool(name="consts", bufs=1))
        wpool = ctx.enter_context(tc.tile_pool(name="wpool", bufs=1))
        xt_pool = ctx.enter_context(tc.tile_pool(name="xt", bufs=6))
        qkstage = ctx.enter_context(tc.tile_pool(name="qkstage", bufs=8))
        tmp_pool = ctx.enter_context(tc.tile_pool(name="tmp", bufs=2))
        vh_pool = ctx.enter_context(tc.tile_pool(name="vh", bufs=1))
        at_pool = ctx.enter_context(tc.tile_pool(name="at", bufs=6))
        small = ctx.enter_context(tc.tile_pool(name="small", bufs=2))
        rb_pool = ctx.enter_context(tc.tile_pool(name="rb", bufs=2))
        ot_pool = ctx.enter_context(tc.tile_pool(name="ot", bufs=2))
        po_pool = ctx.enter_context(tc.tile_pool(name="po", bufs=3))

        # ---- weights on the sync queue interleaved with first x chunks;
        # ---- everything not needed immediately on the scalar HWDGE queue.
        wqk_t = wpool.tile([128, 8, 512], bf16, tag="wqk_t")
        wv_t = wpool.tile([128, 8, 256], bf16, tag="wv_t")
        wp_t = wpool.tile([128, 2, D], bf16, tag="wp_t")
        cos_t = consts.tile([128, T], bf16, tag="cos_t")
        sin_t = consts.tile([128, T], bf16, tag="sin_t")
        tri_t = consts.tile([128, 128], bf16, tag="tri_t")
        bqk_t = consts.tile([128, 4], f32, tag="bqk_t")
        ones64 = consts.tile([1, 64], f32r, tag="ones64")
        sel4 = consts.tile([128, 256], f32r, tag="sel4")
        den128 = consts.tile([128, 512], f32, tag="den128")
        recip128 = consts.tile([128, 512], f32r, tag="recip128")
        nc.vector.memset(den128[:], 1.0)

        for dn in range(8):
            nc.scalar.dma_start(wv_t[:, dn, :], wv[dn * 128:(dn + 1) * 128, :])
        nc.scalar.dma_start(bqk_t[:], bqk)
        nc.scalar.dma_start(cos_t[:], cos4)
        nc.scalar.dma_start(sin_t[:], sin4)
        nc.scalar.dma_start(tri_t[:], trimask)
        nc.scalar.dma_start(ones64[:], ones64_d)
        nc.scalar.dma_start(sel4[:], sel4_d)
        for hd in range(2):
            nc.scalar.dma_start(wp_t[:, hd, :], wp[hd * 128:(hd + 1) * 128, :])

        # v-hat: [128, 16 tok-tiles x (4 heads x 65)]; col 64 of each head = 1.0
        vhat = vh_pool.tile([128, NKT * VW], bf16, tag="vhat")
        vh_ones = vhat[:, :].rearrange("p (t h c) -> p t h c", t=NKT,
                                       h=HPC)[:, :, :, HD:HD + 1]
        nc.scalar.dma_start(vh_ones,
                            ones_pat.rearrange("p (t h) -> p t h", t=NKT)[:, :, :, None])

        # qkv^T output stage tensors [128, T] each
        qE = qkstage.tile([128, T], bf16, tag="qks")
        qO = qkstage.tile([128, T], bf16, tag="qks")
        kE = qkstage.tile([128, T], bf16, tag="qks")
        kO = qkstage.tile([128, T], bf16, tag="qks")
        chunks = [qE, qO, kE, kO]

        # ---- phase 1: qkv matmuls ----
        with tc.tile_pool(name="ps_qk", bufs=4, space="PSUM") as ps_qk, \
             tc.tile_pool(name="ps_v", bufs=4, space="PSUM") as ps_v:
            for tq in range(NTQ):
                t0 = tq * 512
                xc = []
                for dn in range(8):
                    xt = xt_pool.tile([128, 512], bf16, tag="xt")
                    if tq == 0:
                        # interleave weight + activation loads so the first
                        # matmul can start after two small DMAs
                        nc.sync.dma_start(wqk_t[:, dn, :],
                                          wqk[dn * 128:(dn + 1) * 128, :])
                    nc.sync.dma_start(xt[:], xT[dn * 128:(dn + 1) * 128,
                                                t0:t0 + 512])
                    xc.append(xt)
                pqk = [ps_qk.tile([128, 512], f32, tag="ps_qk",
                                  name=f"pqk{tq}_{i}") for i in range(4)]
                pv = [ps_v.tile([128, 256], f32, tag="ps_v",
                                name=f"pv{tq}_{i}") for i in range(4)]
                for dn in range(8):
                    for ch in range(4):
                        nc.tensor.matmul(
                            pqk[ch][:],
                            wqk_t[:, dn, ch * 128:(ch + 1) * 128],
                            xc[dn][:],
                            start=(dn == 0), stop=(dn == 7))
                    for tt in range(4):  # token tiles of 128 within quarter
                        nc.tensor.matmul(
                            pv[tt][:],
                            xc[dn][:, tt * 128:(tt + 1) * 128],
                            wv_t[:, dn, :],
                            start=(dn == 0), stop=(dn == 7))
                # psum -> sbuf copies (+ bias for q,k on ACT)
                for ch in range(4):
                    nc.scalar.activation(
                        chunks[ch][:, t0:t0 + 512], pqk[ch][:], IDENT,
                        bias=bqk_t[:, ch:ch + 1], scale=1.0)
                for tt in range(4):
                    tglob = tq * 4 + tt
                    dst = vhat[:, tglob * VW:(tglob + 1) * VW].rearrange(
                        "p (h c) -> p h c", h=HPC)[:, :, 0:HD]
                    nc.vector.tensor_copy(
                        dst, pv[tt][:].rearrange("p (h c) -> p h c", h=HPC))

        # ---- phase 2: rope + permute ----
        qF = qkstage.tile([128, T], bf16, tag="qks")
        qS = qkstage.tile([128, T], bf16, tag="qks")
        kF = qkstage.tile([128, T], bf16, tag="qks")
        kS = qkstage.tile([128, T], bf16, tag="qks")
        for (E, O, F, S) in ((qE, qO, qF, qS), (kE, kO, kF, kS)):
            for tq in range(NTQ):
                sl = slice(tq * 512, tq * 512 + 512)
                t1 = tmp_pool.tile([128, 512], bf16, tag="tmp")
                t2 = tmp_pool.tile([128, 512], bf16, tag="tmp")
                nc.vector.tensor_mul(t1[:], E[:, sl], cos_t[:, sl])
                nc.vector.tensor_mul(t2[:], O[:, sl], sin_t[:, sl])
                nc.vector.tensor_sub(F[:, sl], t1[:], t2[:])
                t3 = tmp_pool.tile([128, 512], bf16, tag="tmp")
                t4 = tmp_pool.tile([128, 512], bf16, tag="tmp")
                nc.vector.tensor_mul(t3[:], E[:, sl], sin_t[:, sl])
                nc.vector.tensor_mul(t4[:], O[:, sl], cos_t[:, sl])
                nc.vector.tensor_add(S[:, sl], t3[:], t4[:])

        if dbg:
            nc.scalar.dma_start(dbg_qE, qE[:])
            nc.scalar.dma_start(dbg_qF, qF[:])
            nc.scalar.dma_start(dbg_vhat, vhat[:])

        # permute (h-major 32-blocks F/S) -> per-head-64-contiguous qr/kr
        qrA = qkstage.tile([128, T], bf16, tag="qks")
        qrB = qkstage.tile([128, T], bf16, tag="qks")
        krA = qkstage.tile([128, T], bf16, tag="qks")
        krB = qkstage.tile([128, T], bf16, tag="qks")
        for (F, S, rA, rB) in ((qF, qS, qrA, qrB), (kF, kS, krA, krB)):
            for h in range(4):
                dst = rA if h < 2 else rB
                r0 = (h % 2) * 64
                nc.sync.dma_start(dst[r0:r0 + 32, :], F[h * 32:(h + 1) * 32, :])
                nc.sync.dma_start(dst[r0 + 32:r0 + 64, :],
                                  S[h * 32:(h + 1) * 32, :])

        # ---- phase 3: attention (head-outer for stationary reuse) ----
        otA = ot_pool.tile([128, T], bf16, tag="ot")
        otB = ot_pool.tile([128, T], bf16, tag="ot")
        with tc.tile_pool(name="ps_s", bufs=4, space="PSUM") as ps_s, \
             tc.tile_pool(name="ps_o", bufs=4, space="PSUM") as ps_o:
            for h in range(HPC):
                qr = qrA if h < 2 else qrB
                kr = krA if h < 2 else krB
                r0 = (h % 2) * 64
                po = [ps_o.tile([65, 512], f32, tag="ps_o",
                                name=f"po{h}_{i}") for i in range(NQC)]
                for kt in range(NKT):
                    k0 = kt * 128
                    qc_lo = kt // 4  # first query chunk attending this k-tile
                    ats = {}
                    for qc in range(qc_lo, NQC):
                        q0 = qc * 512
                        col_lo = k0 - q0 if k0 > q0 else 0  # diag sub-range
                        pss = ps_s.tile([128, 512], f32, tag="ps_s",
                                        name=f"pss{h}_{kt}_{qc}")
                        nc.tensor.matmul(
                            pss[:, col_lo:512],
                            kr[r0:r0 + 64, k0:k0 + 128],
                            qr[r0:r0 + 64, q0 + col_lo:q0 + 512],
                            start=True, stop=True)
                        at = at_pool.tile([128, 512], bf16, tag="at",
                                          name=f"at{h}_{kt}_{qc}")
                        nc.scalar.activation(at[:, col_lo:512],
                                             pss[:, col_lo:512], EXP)
                        if qc == qc_lo and k0 >= q0:
                            nc.vector.tensor_mul(
                                at[:, col_lo:col_lo + 128],
                                at[:, col_lo:col_lo + 128], tri_t[:])
                        ats[qc] = (at, col_lo)
                        if dbg and h == 0 and kt == 0 and qc == 0:
                            nc.scalar.dma_start(dbg_at, at[:])
                    for qc in range(qc_lo, NQC):
                        at, col_lo = ats[qc]
                        nc.tensor.matmul(
                            po[qc][:, col_lo:512],
                            vhat[:, kt * VW + h * 65:kt * VW + (h + 1) * 65],
                            at[:, col_lo:512],
                            start=(kt == 0), stop=(kt == 4 * qc + 3),
                            skip_group_check=True)
                if dbg and h == 0:
                    po_dump = rb_pool.tile([65, 512], f32, tag="podump")
                    nc.scalar.copy(po_dump[:], po[0][:])
                    nc.scalar.dma_start(dbg_po, po_dump[:])
                # normalize: gather the 4 denom rows to 32-aligned partitions
                # of den128 (other rows stay 1.0 from the init memset), one
                # fast approx-recip over the whole tile, broadcast each row
                # via a one-hot PE matmul, multiply out of PSUM
                for qc in range(NQC):
                    nc.vector.tensor_copy(den128[32 * qc:32 * qc + 1, :],
                                          po[qc][64:65, :])
                c = RECIP_APPROX_FAST_CONSTS
                nc.vector._custom_dve(
                    RECIPROCAL_APPROX_FAST, out=recip128[:], in0=den128[:],
                    s0=c["s0"], s1=c["s1"], imm2=c["imm2"])
                for qc in range(NQC):
                    q0 = qc * 512
                    prb = ps_s.tile([64, 512], f32, tag="ps_s",
                                    name=f"prb{h}_{qc}")
                    nc.tensor.matmul(prb[:],
                                     sel4[:, qc * 64:(qc + 1) * 64],
                                     recip128[:], start=True, stop=True)
                    rb = rb_pool.tile([64, 512], f32, tag="rb",
                                      name=f"rb{h}_{qc}")
                    nc.scalar.copy(rb[:], prb[:])
                    ot = otA if h < 2 else otB
                    nc.vector.tensor_mul(ot[r0:r0 + 64, q0:q0 + 512],
                                         po[qc][0:64, :], rb[:])

            if dbg:
                nc.scalar.dma_start(dbg_qrA, qrA[:])
                nc.scalar.dma_start(dbg_ot, otA[:])

            # ---- phase 4: projection (stationary reuse across qc) ----
            for oc in range(8):
                pj = [ps_s.tile([128, 512], f32, tag="ps_s",
                                name=f"pj{oc}_{i}") for i in range(NQC)]
                for hd in range(2):
                    src = otA if hd == 0 else otB
                    for qc in range(NQC):
                        nc.tensor.matmul(
                            pj[qc][:], wp_t[:, hd, oc * 128:(oc + 1) * 128],
                            src[:, qc * 512:qc * 512 + 512],
                            start=(hd == 0), stop=(hd == 1))
                for qc in range(NQC):
                    ob = po_pool.tile([128, 512], bf16, tag="po",
                                      name=f"ob{oc}_{qc}")
                    nc.any.tensor_copy(ob[:], pj[qc][:])
                    nc.sync.dma_start(
                        outT[oc * 128:(oc + 1) * 128,
                             qc * 512:qc * 512 + 512], ob[:])

    nc.compile()
    return nc


_NC = None


def _get_nc():
    global _NC
    if _NC is None:
        _NC = _build()
    return _NC


def _sel4():
    """[128, 256] one-hot stationary: column qc*64+p reads partition 32*qc."""
    s = np.zeros((128, 256), dtype=np.float32)
    for qc in range(4):
        s[32 * qc, qc * 64:(qc + 1) * 64] = 1.0
    return s


def _host_prep(x, Wqkv, bqkv, Wproj, bproj, pos):
    """Build the 8 per-core input maps."""
    import ml_dtypes
    nbf16 = ml_dtypes.bfloat16

    x = np.asarray(x, dtype=np.float32)
    Wqkv = np.asarray(Wqkv, dtype=np.float32)
    bqkv = np.asarray(bqkv, dtype=np.float32)
    Wproj = np.asarray(Wproj, dtype=np.float32)
    bproj = np.asarray(bproj, dtype=np.float32)
    pos = int(np.asarray(pos))

    scale = HD ** -0.5
    # rope tables, layout [128 = 4 heads x 32 thetas (h-major), T]
    theta = 1.0 / BASE ** (np.arange(HALF, dtype=np.float32) / HALF)
    angles = np.outer(np.arange(pos, pos + T, dtype=np.float32), theta)  # [T,32]
    cosT = np.cos(angles).T.astype(np.float32)  # [32, T]
    sinT = np.sin(angles).T.astype(np.float32)
    cos4 = np.ascontiguousarray(np.tile(cosT, (4, 1))).astype(nbf16)  # [128, T]
    sin4 = np.ascontiguousarray(np.tile(sinT, (4, 1))).astype(nbf16)

    tri = np.tril(np.ones((128, 128), dtype=np.float32)).T  # m[p,j]=1 if p<=j
    tri = np.ascontiguousarray(tri).astype(nbf16)

    in_maps = []
    for c in range(8):
        b, hg = c // 4, c % 4
        heads = [4 * hg + h for h in range(HPC)]
        permE = np.array([h * HD + 2 * i for h in heads for i in range(HALF)])
        permO = permE + 1
        wqk_np = np.concatenate([
            Wqkv[:, permE] * scale,          # qE
            Wqkv[:, permO] * scale,          # qO
            Wqkv[:, D + permE],              # kE
            Wqkv[:, D + permO],              # kO
        ], axis=1)
        bqk_np = np.stack([
            bqkv[permE] * scale, bqkv[permO] * scale,
            bqkv[D + permE], bqkv[D + permO],
        ], axis=1)
        wv_np = Wqkv[:, 2 * D + 256 * hg: 2 * D + 256 * (hg + 1)]
        wp_np = Wproj[256 * hg: 256 * (hg + 1), :]
        in_maps.append({
            "xT": np.ascontiguousarray(x[b].T).astype(nbf16),
            "wqk": np.ascontiguousarray(wqk_np).astype(nbf16),
            "wv": np.ascontiguousarray(wv_np).astype(nbf16),
            "wp": np.ascontiguousarray(wp_np).astype(nbf16),
            "bqk": np.ascontiguousarray(bqk_np, dtype=np.float32),
            "cos4": cos4,
            "sin4": sin4,
            "trimask": tri,
            "ones64_d": np.ones((1, 64), dtype=np.float32),
            "sel4_d": _sel4(),
            "ones_pat": np.ones((128, 64), dtype=nbf16),
        })
    const_vec = bqkv[2 * D:] @ Wproj + bproj  # exact host-side bias handling
    return in_maps, const_vec


def kernel(x, Wqkv, bqkv, Wproj, bproj, pos, **kw):
    in_maps, const_vec = _host_prep(x, Wqkv, bqkv, Wproj, bproj, pos)
    nc = _get_nc()
    res = run_bass_kernel_spmd(nc, in_maps, core_ids=list(range(8))).results
    out = np.empty((B, T, D), dtype=np.float32)
    for b in range(B):
        acc = res[4 * b]["outT"].astype(np.float32)
        for c in range(4 * b + 1, 4 * b + 4):
            acc += res[c]["outT"].astype(np.float32)
        out[b] = acc.T + const_vec
    return out


# revision 34
# speedup vs baseline: 1.2655x; 1.0075x over previous
"""Multi-head self-attention (RoPE, causal) Trainium2 Bass kernel.

Full inputs in, full output out. Sharding: 8 cores = 2 batch x 4 head-groups
(4 heads each). Per core: qkv projection, RoPE on DVE, streaming causal
attention (S^T orientation: softmax reduction along partitions via a
ones-column in V-hat), output projection partial. Host sums the 4 per-batch
partials and adds the (bv @ Wproj + bproj) constant.

v2: all matmul operands in bf16 (PSUM accumulation stays fp32) — lower PE
power (avoids the sustained K=4/8 clock throttle seen with f32r), fast
weight loads, half the DMA bytes. Reciprocal via the fast custom-DVE
approximation. RoPE in bf16 for DVE 2x throughput. Output staged bf16.

Self-contained: hardcodes all shapes for B=2, T=2048, D=1024, H=16, hd=64.
"""
from contextlib import ExitStack

import numpy as np

from concourse import bacc, mybir, tile
from concourse.bass_utils import run_bass_kernel_spmd
from concourse.dve_ops import RECIP_APPROX_FAST_CONSTS, RECIPROCAL_APPROX_FAST

f32 = mybir.dt.float32
f32r = mybir.dt.float32r
bf16 = mybir.dt.bfloat16
EXP = mybir.ActivationFunctionType.Exp
IDENT = mybir.ActivationFunctionType.Identity

B, T, D = 2, 2048, 1024
H, HD = 16, 64
HALF = HD // 2  # 32
HPC = 4  # heads per core
BASE = 10000.0
NTQ = 4  # token quarters of 512 (qkv phase)
NQC = 4  # query chunks of 512 (attention phase)
NKT = 16  # key tiles of 128
VW = HPC * (HD + 1)  # 260: v-hat columns per token tile


def _build(dbg=False):
    nc = bacc.Bacc("TRN2", target_bir_lowering=False, debug=False, num_devices=8)

    xT = nc.dram_tensor("xT", [D, T], bf16, kind="ExternalInput").ap()
    wqk = nc.dram_tensor("wqk", [D, 512], bf16, kind="ExternalInput").ap()
    wv = nc.dram_tensor("wv", [D, 256], bf16, kind="ExternalInput").ap()
    wp = nc.dram_tensor("wp", [256, D], bf16, kind="ExternalInput").ap()
    bqk = nc.dram_tensor("bqk", [128, 4], f32, kind="ExternalInput").ap()
    cos4 = nc.dram_tensor("cos4", [128, T], bf16, kind="ExternalInput").ap()
    sin4 = nc.dram_tensor("sin4", [128, T], bf16, kind="ExternalInput").ap()
    trimask = nc.dram_tensor("trimask", [128, 128], bf16, kind="ExternalInput").ap()
    ones64_d = nc.dram_tensor("ones64_d", [1, 64], f32r, kind="ExternalInput").ap()
    ones_pat = nc.dram_tensor("ones_pat", [128, 64], bf16, kind="ExternalInput").ap()
    outT = nc.dram_tensor("outT", [D, T], bf16, kind="ExternalOutput").ap()
    if dbg:
        dbg_qE = nc.dram_tensor("dbg_qE", [128, T], bf16, kind="ExternalOutput").ap()
        dbg_qF = nc.dram_tensor("dbg_qF", [128, T], bf16, kind="ExternalOutput").ap()
        dbg_qrA = nc.dram_tensor("dbg_qrA", [128, T], bf16, kind="ExternalOutput").ap()
        dbg_vhat = nc.dram_tensor("dbg_vhat", [128, NKT * VW], bf16, kind="ExternalOutput").ap()
        dbg_at = nc.dram_tensor("dbg_at", [128, 512], bf16, kind="ExternalOutput").ap()
        dbg_po = nc.dram_tensor("dbg_po", [65, 512], mybir.dt.float32, kind="ExternalOutput").ap()
        dbg_ot = nc.dram_tensor("dbg_ot", [128, T], bf16, kind="ExternalOutput").ap()

    with tile.TileContext(nc) as tc, ExitStack() as ctx:
        consts = ctx.enter_context(tc.tile_pool(name="consts", bufs=1))
        wpool = ctx.enter_context(tc.tile_pool(name="wpool", bufs=1))
        xt_pool = ctx.enter_context(tc.tile_pool(name="xt", bufs=10))
        qkstage = ctx.enter_context(tc.tile_pool(name="qkstage", bufs=8))
        tmp_pool = ctx.enter_context(tc.tile_pool(name="tmp", bufs=2))
        vh_pool = ctx.enter_context(tc.tile_pool(name="vh", bufs=1))
        at_pool = ctx.enter_context(tc.tile_pool(name="at", bufs=10))
        small = ctx.enter_context(tc.tile_pool(name="small", bufs=8))
        rb_pool = ctx.enter_context(tc.tile_pool(name="rb", bufs=2))
        ot_pool = ctx.enter_context(tc.tile_pool(name="ot", bufs=2))
        po_pool = ctx.enter_context(tc.tile_pool(name="po", bufs=3))

        # ---- weights on the sync queue interleaved with first x chunks;
        # ---- everything not needed immediately on the scalar HWDGE queue.
        wqk_t = wpool.tile([128, 8, 512], bf16, tag="wqk_t")
        wv_t = wpool.tile([128, 8, 256], bf16, tag="wv_t")
        wp_t = wpool.tile([128, 2, D], bf16, tag="wp_t")
        cos_t = consts.tile([128, T], bf16, tag="cos_t")
        sin_t = consts.tile([128, T], bf16, tag="sin_t")
        tri_t = consts.tile([128, 128], bf16, tag="tri_t")
        bqk_t = consts.tile([128, 4], f32, tag="bqk_t")
        ones64 = consts.tile([1, 64], f32r, tag="ones64")


        for dn in range(8):
            nc.scalar.dma_start(wv_t[:, dn, :], wv[dn * 128:(dn + 1) * 128, :])
        nc.scalar.dma_start(bqk_t[:], bqk)
        nc.scalar.dma_start(cos_t[:], cos4)
        nc.scalar.dma_start(sin_t[:], sin4)
        nc.scalar.dma_start(tri_t[:], trimask)
        nc.scalar.dma_start(ones64[:], ones64_d)
        for hd in range(2):
            nc.scalar.dma_start(wp_t[:, hd, :], wp[hd * 128:(hd + 1) * 128, :])

        # v-hat: [128, 16 tok-tiles x (4 heads x 65)]; col 64 of each head = 1.0
        vhat = vh_pool.tile([128, NKT * VW], bf16, tag="vhat")
        vh_ones = vhat[:, :].rearrange("p (t h c) -> p t h c", t=NKT,
                                       h=HPC)[:, :, :, HD:HD + 1]
        nc.scalar.dma_start(vh_ones,
                            ones_pat.rearrange("p (t h) -> p t h", t=NKT)[:, :, :, None])

        # qkv^T output stage tensors [128, T] each
        qE = qkstage.tile([128, T], bf16, tag="qks")
        qO = qkstage.tile([128, T], bf16, tag="qks")
        kE = qkstage.tile([128, T], bf16, tag="qks")
        kO = qkstage.tile([128, T], bf16, tag="qks")
        chunks = [qE, qO, kE, kO]

        # ---- phase 1: qkv matmuls ----
        with tc.tile_pool(name="ps_qk", bufs=4, space="PSUM") as ps_qk, \
             tc.tile_pool(name="ps_v", bufs=4, space="PSUM") as ps_v:
            for tq in range(NTQ):
                t0 = tq * 512
                xc = []
                for dn in range(8):
                    xt = xt_pool.tile([128, 512], bf16, tag="xt")
                    if tq == 0:
                        # interleave weight + activation loads so the first
                        # matmul can start after two small DMAs
                        nc.sync.dma_start(wqk_t[:, dn, :],
                                          wqk[dn * 128:(dn + 1) * 128, :])
                    # alternate x chunks between both HWDGE queues
                    eng = nc.sync if dn % 2 == 0 else nc.scalar
                    eng.dma_start(xt[:], xT[dn * 128:(dn + 1) * 128,
                                            t0:t0 + 512])
                    xc.append(xt)
                pqk = [ps_qk.tile([128, 512], f32, tag="ps_qk",
                                  name=f"pqk{tq}_{i}") for i in range(4)]
                pv = [ps_v.tile([128, 256], f32, tag="ps_v",
                                name=f"pv{tq}_{i}") for i in range(4)]
                for dn in range(8):
                    for ch in range(4):
                        nc.tensor.matmul(
                            pqk[ch][:],
                            wqk_t[:, dn, ch * 128:(ch + 1) * 128],
                            xc[dn][:],
                            start=(dn == 0), stop=(dn == 7))
                    for tt in range(4):  # token tiles of 128 within quarter
                        nc.tensor.matmul(
                            pv[tt][:],
                            xc[dn][:, tt * 128:(tt + 1) * 128],
                            wv_t[:, dn, :],
                            start=(dn == 0), stop=(dn == 7))
                # psum -> sbuf copies (+ bias for q,k on ACT)
                for ch in range(4):
                    nc.scalar.activation(
                        chunks[ch][:, t0:t0 + 512], pqk[ch][:], IDENT,
                        bias=bqk_t[:, ch:ch + 1], scale=1.0)
                for tt in range(4):
                    tglob = tq * 4 + tt
                    dst = vhat[:, tglob * VW:(tglob + 1) * VW].rearrange(
                        "p (h c) -> p h c", h=HPC)[:, :, 0:HD]
                    nc.vector.tensor_copy(
                        dst, pv[tt][:].rearrange("p (h c) -> p h c", h=HPC))

        # ---- phase 2: rope + permute ----
        qF = qkstage.tile([128, T], bf16, tag="qks")
        qS = qkstage.tile([128, T], bf16, tag="qks")
        kF = qkstage.tile([128, T], bf16, tag="qks")
        kS = qkstage.tile([128, T], bf16, tag="qks")
        for (E, O, F, S) in ((qE, qO, qF, qS), (kE, kO, kF, kS)):
            for tq in range(NTQ):
                sl = slice(tq * 512, tq * 512 + 512)
                t1 = tmp_pool.tile([128, 512], bf16, tag="tmp")
                t2 = tmp_pool.tile([128, 512], bf16, tag="tmp")
                nc.vector.tensor_mul(t1[:], E[:, sl], cos_t[:, sl])
                nc.vector.tensor_mul(t2[:], O[:, sl], sin_t[:, sl])
                nc.vector.tensor_sub(F[:, sl], t1[:], t2[:])
                t3 = tmp_pool.tile([128, 512], bf16, tag="tmp")
                t4 = tmp_pool.tile([128, 512], bf16, tag="tmp")
                nc.vector.tensor_mul(t3[:], E[:, sl], sin_t[:, sl])
                nc.vector.tensor_mul(t4[:], O[:, sl], cos_t[:, sl])
                nc.vector.tensor_add(S[:, sl], t3[:], t4[:])

        if dbg:
            nc.scalar.dma_start(dbg_qE, qE[:])
            nc.scalar.dma_start(dbg_qF, qF[:])
            nc.scalar.dma_start(dbg_vhat, vhat[:])

        # permute (h-major 32-blocks F/S) -> per-head-64-contiguous qr/kr
        qrA = qkstage.tile([128, T], bf16, tag="qks")
        qrB = qkstage.tile([128, T], bf16, tag="qks")
        krA = qkstage.tile([128, T], bf16, tag="qks")
        krB = qkstage.tile([128, T], bf16, tag="qks")
        for (F, S, rA, rB) in ((qF, qS, qrA, qrB), (kF, kS, krA, krB)):
            for h in range(4):
                dst = rA if h < 2 else rB
                r0 = (h % 2) * 64
                nc.sync.dma_start(dst[r0:r0 + 32, :], F[h * 32:(h + 1) * 32, :])
                nc.sync.dma_start(dst[r0 + 32:r0 + 64, :],
                                  S[h * 32:(h + 1) * 32, :])

        # ---- phase 3: attention (head-outer for stationary reuse) ----
        # PV matmuls run one k-tile behind the S matmuls so the exp (ACT)
        # latency is hidden behind PE work and the PE never micro-stalls
        # (keeps the HAM clock gate at 8/8).
        otA = ot_pool.tile([128, T], bf16, tag="ot")
        otB = ot_pool.tile([128, T], bf16, tag="ot")
        with tc.tile_pool(name="ps_s", bufs=4, space="PSUM") as ps_s, \
             tc.tile_pool(name="ps_o", bufs=4, space="PSUM") as ps_o:
            for h in range(HPC):
                qr = qrA if h < 2 else qrB
                kr = krA if h < 2 else krB
                r0 = (h % 2) * 64
                po = [ps_o.tile([65, 512], f32, tag="ps_o",
                                name=f"po{h}_{i}") for i in range(NQC)]
                dens = [small.tile([1, 512], f32, tag="den",
                                   name=f"den{h}_{i}") for i in range(NQC)]

                def issue_s(kt):
                    k0 = kt * 128
                    qc_lo = kt // 4
                    ats = []
                    for qc in range(qc_lo, NQC):
                        q0 = qc * 512
                        col_lo = k0 - q0 if k0 > q0 else 0  # diag sub-range
                        pss = ps_s.tile([128, 512], f32, tag="ps_s",
                                        name=f"pss{h}_{kt}_{qc}")
                        nc.tensor.matmul(
                            pss[:, col_lo:512],
                            kr[r0:r0 + 64, k0:k0 + 128],
                            qr[r0:r0 + 64, q0 + col_lo:q0 + 512],
                            start=True, stop=True)
                        at = at_pool.tile([128, 512], bf16, tag="at",
                                          name=f"at{h}_{kt}_{qc}")
                        nc.scalar.activation(at[:, col_lo:512],
                                             pss[:, col_lo:512], EXP)
                        if qc == qc_lo and k0 >= q0:
                            nc.vector.tensor_mul(
                                at[:, col_lo:col_lo + 128],
                                at[:, col_lo:col_lo + 128], tri_t[:])
                        ats.append((qc, at, col_lo))
                        if dbg and h == 0 and kt == 0 and qc == 0:
                            nc.scalar.dma_start(dbg_at, at[:])
                    return ats

                def issue_pv(kt, ats):
                    for qc, at, col_lo in ats:
                        nc.tensor.matmul(
                            po[qc][:, col_lo:512],
                            vhat[:, kt * VW + h * 65:kt * VW + (h + 1) * 65],
                            at[:, col_lo:512],
                            start=(kt == 0), stop=(kt == 4 * qc + 3),
                            skip_group_check=True)
                        if kt == 4 * qc + 3:
                            # denominator row final: gather to a base-0 SBUF
                            # tile while the kt loop streams on
                            nc.vector.tensor_copy(dens[qc][:],
                                                  po[qc][64:65, :])

                prev = None
                for kt in range(NKT):
                    ats = issue_s(kt)
                    if prev is not None:
                        issue_pv(kt - 1, prev)
                    prev = ats
                issue_pv(NKT - 1, prev)

                if dbg and h == 0:
                    po_dump = rb_pool.tile([65, 512], f32, tag="podump")
                    nc.scalar.copy(po_dump[:], po[0][:])
                    nc.scalar.dma_start(dbg_po, po_dump[:])
                # normalize off the PE/ACT path: fast approx-recip per chunk,
                # GpSimd partition-broadcast, DVE multiply out of PSUM
                c = RECIP_APPROX_FAST_CONSTS
                for qc in range(NQC):
                    q0 = qc * 512
                    recip = small.tile([1, 512], f32, tag="recip",
                                       name=f"recip{h}_{qc}")
                    nc.vector._custom_dve(
                        RECIPROCAL_APPROX_FAST, out=recip[:],
                        in0=dens[qc][:], s0=c["s0"], s1=c["s1"],
                        imm2=c["imm2"])
                    rb = rb_pool.tile([64, 512], f32, tag="rb",
                                      name=f"rb{h}_{qc}")
                    nc.gpsimd.partition_broadcast(rb[:], recip[:])
                    ot = otA if h < 2 else otB
                    nc.vector.tensor_mul(ot[r0:r0 + 64, q0:q0 + 512],
                                         po[qc][0:64, :], rb[:])

            if dbg:
                nc.scalar.dma_start(dbg_qrA, qrA[:])
                nc.scalar.dma_start(dbg_ot, otA[:])

            # ---- phase 4: projection (stationary reuse across qc) ----
            for oc in range(8):
                pj = [ps_s.tile([128, 512], f32, tag="ps_s",
                                name=f"pj{oc}_{i}") for i in range(NQC)]
                for hd in range(2):
                    src = otA if hd == 0 else otB
                    for qc in range(NQC):
                        nc.tensor.matmul(
                            pj[qc][:], wp_t[:, hd, oc * 128:(oc + 1) * 128],
                            src[:, qc * 512:qc * 512 + 512],
                            start=(hd == 0), stop=(hd == 1))
                for qc in range(NQC):
                    ob = po_pool.tile([128, 512], bf16, tag="po",
                                      name=f"ob{oc}_{qc}")
                    nc.any.tensor_copy(ob[:], pj[qc][:])
                    nc.sync.dma_start(
                        outT[oc * 128:(oc + 1) * 128,
                             qc * 512:qc * 512 + 512], ob[:])

    nc.compile()
    return nc


_NC = None


def _get_nc():
    global _NC
    if _NC is None:
        _NC = _build()
    return _NC


def _sel4():
    """[128, 256] one-hot stationary: column qc*64+p reads partition 32*qc."""
    s = np.zeros((128, 256), dtype=np.float32)
    for qc in range(4):
        s[32 * qc, qc * 64:(qc + 1) * 64] = 1.0
    return s


def _host_prep(x, Wqkv, bqkv, Wproj, bproj, pos):
    """Build the 8 per-core input maps."""
    import ml_dtypes
    nbf16 = ml_dtypes.bfloat16

    x = np.asarray(x, dtype=np.float32)
    Wqkv = np.asarray(Wqkv, dtype=np.float32)
    bqkv = np.asarray(bqkv, dtype=np.float32)
    Wproj = np.asarray(Wproj, dtype=np.float32)
    bproj = np.asarray(bproj, dtype=np.float32)
    pos = int(np.asarray(pos))

    scale = HD ** -0.5
    # rope tables, layout [128 = 4 heads x 32 thetas (h-major), T]
    theta = 1.0 / BASE ** (np.arange(HALF, dtype=np.float32) / HALF)
    angles = np.outer(np.arange(pos, pos + T, dtype=np.float32), theta)  # [T,32]
    cosT = np.cos(angles).T.astype(np.float32)  # [32, T]
    sinT = np.sin(angles).T.astype(np.float32)
    cos4 = np.ascontiguousarray(np.tile(cosT, (4, 1))).astype(nbf16)  # [128, T]
    sin4 = np.ascontiguousarray(np.tile(sinT, (4, 1))).astype(nbf16)

    tri = np.tril(np.ones((128, 128), dtype=np.float32)).T  # m[p,j]=1 if p<=j
    tri = np.ascontiguousarray(tri).astype(nbf16)

    in_maps = []
    for c in range(8):
        b, hg = c // 4, c % 4
        heads = [4 * hg + h for h in range(HPC)]
        permE = np.array([h * HD + 2 * i for h in heads for i in range(HALF)])
        permO = permE + 1
        wqk_np = np.concatenate([
            Wqkv[:, permE] * scale,          # qE
            Wqkv[:, permO] * scale,          # qO
            Wqkv[:, D + permE],              # kE
            Wqkv[:, D + permO],              # kO
        ], axis=1)
        bqk_np = np.stack([
            bqkv[permE] * scale, bqkv[permO] * scale,
            bqkv[D + permE], bqkv[D + permO],
        ], axis=1)
        wv_np = Wqkv[:, 2 * D + 256 * hg: 2 * D + 256 * (hg + 1)]
        wp_np = Wproj[256 * hg: 256 * (hg + 1), :]
        in_maps.append({
            "xT": np.ascontiguousarray(x[b].T).astype(nbf16),
            "wqk": np.ascontiguousarray(wqk_np).astype(nbf16),
            "wv": np.ascontiguousarray(wv_np).astype(nbf16),
            "wp": np.ascontiguousarray(wp_np).astype(nbf16),
            "bqk": np.ascontiguousarray(bqk_np, dtype=np.float32),
            "cos4": cos4,
            "sin4": sin4,
            "trimask": tri,
            "ones64_d": np.ones((1, 64), dtype=np.float32),

            "ones_pat": np.ones((128, 64), dtype=nbf16),
        })
    const_vec = bqkv[2 * D:] @ Wproj + bproj  # exact host-side bias handling
    return in_maps, const_vec


def kernel(x, Wqkv, bqkv, Wproj, bproj, pos, **kw):
    in_maps, const_vec = _host_prep(x, Wqkv, bqkv, Wproj, bproj, pos)
    nc = _get_nc()
    res = run_bass_kernel_spmd(nc, in_maps, core_ids=list(range(8))).results
    out = np.empty((B, T, D), dtype=np.float32)
    for b in range(B):
        acc = res[4 * b]["outT"].astype(np.float32)
        for c in range(4 * b + 1, 4 * b + 4):
            acc += res[c]["outT"].astype(np.float32)
        out[b] = acc.T + const_vec
    return out


# revision 40
# speedup vs baseline: 1.3912x; 1.0994x over previous
"""Multi-head self-attention (RoPE, causal) Trainium2 Bass kernel.

Full inputs in, full output out. Sharding: 8 cores = 2 batch x 4 head-groups
(4 heads each). Per core: qkv projection, RoPE on DVE, streaming causal
attention (S^T orientation: softmax reduction along partitions via a
ones-column in V-hat), output projection partial. Host sums the 4 per-batch
partials and adds the (bv @ Wproj + bproj) constant.

v2: all matmul operands in bf16 (PSUM accumulation stays fp32) — lower PE
power (avoids the sustained K=4/8 clock throttle seen with f32r), fast
weight loads, half the DMA bytes. Reciprocal via the fast custom-DVE
approximation. RoPE in bf16 for DVE 2x throughput. Output staged bf16.

Self-contained: hardcodes all shapes for B=2, T=2048, D=1024, H=16, hd=64.
"""
from contextlib import ExitStack

import numpy as np

from concourse import bacc, mybir, tile
from concourse.bass_utils import run_bass_kernel_spmd
from concourse.dve_ops import RECIP_APPROX_FAST_CONSTS, RECIPROCAL_APPROX_FAST

f32 = mybir.dt.float32
f32r = mybir.dt.float32r
bf16 = mybir.dt.bfloat16
EXP = mybir.ActivationFunctionType.Exp
IDENT = mybir.ActivationFunctionType.Identity

B, T, D = 2, 2048, 1024
H, HD = 16, 64
HALF = HD // 2  # 32
HPC = 4  # heads per core
BASE = 10000.0
NTQ = 4  # token quarters of 512 (qkv phase)
NQC = 4  # query chunks of 512 (attention phase)
NKT = 16  # key tiles of 128
VW = HPC * (HD + 1)  # 260: v-hat columns per token tile


def _build(dbg=False):
    nc = bacc.Bacc("TRN2", target_bir_lowering=False, debug=False, num_devices=8)

    xT = nc.dram_tensor("xT", [D, T], bf16, kind="ExternalInput").ap()
    wqk = nc.dram_tensor("wqk", [D, 512], bf16, kind="ExternalInput").ap()
    wv = nc.dram_tensor("wv", [D, 256], bf16, kind="ExternalInput").ap()
    wp = nc.dram_tensor("wp", [256, D], bf16, kind="ExternalInput").ap()
    bqk = nc.dram_tensor("bqk", [128, 4], f32, kind="ExternalInput").ap()
    cos4 = nc.dram_tensor("cos4", [128, T], bf16, kind="ExternalInput").ap()
    sin4 = nc.dram_tensor("sin4", [128, T], bf16, kind="ExternalInput").ap()
    trimask = nc.dram_tensor("trimask", [128, 128], bf16, kind="ExternalInput").ap()
    ones64_d = nc.dram_tensor("ones64_d", [1, 64], f32r, kind="ExternalInput").ap()
    ones_pat = nc.dram_tensor("ones_pat", [128, 64], bf16, kind="ExternalInput").ap()
    outT = nc.dram_tensor("outT", [D, T], bf16, kind="ExternalOutput").ap()
    if dbg:
        dbg_qE = nc.dram_tensor("dbg_qE", [128, T], bf16, kind="ExternalOutput").ap()
        dbg_qF = nc.dram_tensor("dbg_qF", [128, T], bf16, kind="ExternalOutput").ap()
        dbg_qrA = nc.dram_tensor("dbg_qrA", [128, T], bf16, kind="ExternalOutput").ap()
        dbg_vhat = nc.dram_tensor("dbg_vhat", [128, NKT * VW], bf16, kind="ExternalOutput").ap()
        dbg_at = nc.dram_tensor("dbg_at", [128, 512], bf16, kind="ExternalOutput").ap()
        dbg_po = nc.dram_tensor("dbg_po", [65, 512], mybir.dt.float32, kind="ExternalOutput").ap()
        dbg_ot = nc.dram_tensor("dbg_ot", [128, T], bf16, kind="ExternalOutput").ap()

    with tile.TileContext(nc) as tc, ExitStack() as ctx:
        consts = ctx.enter_context(tc.tile_pool(name="consts", bufs=1))
        wpool = ctx.enter_context(tc.tile_pool(name="wpool", bufs=1))
        xt_pool = ctx.enter_context(tc.tile_pool(name="xt", bufs=10))
        qkstage = ctx.enter_context(tc.tile_pool(name="qkstage", bufs=12))
        tmp_pool = ctx.enter_context(tc.tile_pool(name="tmp", bufs=2))
        vh_pool = ctx.enter_context(tc.tile_pool(name="vh", bufs=1))
        at_pool = ctx.enter_context(tc.tile_pool(name="at", bufs=10))
        small = ctx.enter_context(tc.tile_pool(name="small", bufs=8))
        rb_pool = ctx.enter_context(tc.tile_pool(name="rb", bufs=2))
        ot_pool = ctx.enter_context(tc.tile_pool(name="ot", bufs=2))
        po_pool = ctx.enter_context(tc.tile_pool(name="po", bufs=6))

        # ---- weights on the sync queue interleaved with first x chunks;
        # ---- everything not needed immediately on the scalar HWDGE queue.
        wqk_t = wpool.tile([128, 8, 512], bf16, tag="wqk_t")
        wv_t = wpool.tile([128, 8, 256], bf16, tag="wv_t")
        wp_t = wpool.tile([128, 2, D], bf16, tag="wp_t")
        cos_t = consts.tile([128, T], bf16, tag="cos_t")
        sin_t = consts.tile([128, T], bf16, tag="sin_t")
        tri_t = consts.tile([128, 128], bf16, tag="tri_t")
        bqk_t = consts.tile([128, 4], f32, tag="bqk_t")
        ones64 = consts.tile([1, 64], f32r, tag="ones64")


        for dn in range(8):
            nc.scalar.dma_start(wv_t[:, dn, :], wv[dn * 128:(dn + 1) * 128, :])
        nc.scalar.dma_start(bqk_t[:], bqk)
        nc.scalar.dma_start(cos_t[:], cos4)
        nc.scalar.dma_start(sin_t[:], sin4)
        nc.scalar.dma_start(tri_t[:], trimask)
        nc.scalar.dma_start(ones64[:], ones64_d)
        for hd in range(2):
            nc.scalar.dma_start(wp_t[:, hd, :], wp[hd * 128:(hd + 1) * 128, :])

        # v-hat: [128, 16 tok-tiles x (4 heads x 65)]; col 64 of each head = 1.0
        vhat = vh_pool.tile([128, NKT * VW], bf16, tag="vhat")
        vh_ones = vhat[:, :].rearrange("p (t h c) -> p t h c", t=NKT,
                                       h=HPC)[:, :, :, HD:HD + 1]
        nc.scalar.dma_start(vh_ones,
                            ones_pat.rearrange("p (t h) -> p t h", t=NKT)[:, :, :, None])

        # qkv^T output stage tensors [128, T] each
        qE = qkstage.tile([128, T], bf16, tag="qks")
        qO = qkstage.tile([128, T], bf16, tag="qks")
        kE = qkstage.tile([128, T], bf16, tag="qks")
        kO = qkstage.tile([128, T], bf16, tag="qks")
        chunks = [qE, qO, kE, kO]
        qF = qkstage.tile([128, T], bf16, tag="qks")
        qS = qkstage.tile([128, T], bf16, tag="qks")
        kF = qkstage.tile([128, T], bf16, tag="qks")
        kS = qkstage.tile([128, T], bf16, tag="qks")
        qrA = qkstage.tile([128, T], bf16, tag="qks")
        qrB = qkstage.tile([128, T], bf16, tag="qks")
        krA = qkstage.tile([128, T], bf16, tag="qks")
        krB = qkstage.tile([128, T], bf16, tag="qks")

        # ---- phase 1: qkv matmuls ----
        with tc.tile_pool(name="ps_qk", bufs=4, space="PSUM") as ps_qk, \
             tc.tile_pool(name="ps_v", bufs=4, space="PSUM") as ps_v:
            for tq in range(NTQ):
                t0 = tq * 512
                xc = []
                for dn in range(8):
                    xt = xt_pool.tile([128, 512], bf16, tag="xt")
                    if tq == 0:
                        # interleave weight + activation loads so the first
                        # matmul can start after two small DMAs
                        nc.sync.dma_start(wqk_t[:, dn, :],
                                          wqk[dn * 128:(dn + 1) * 128, :])
                    # alternate x chunks between both HWDGE queues
                    eng = nc.sync if dn % 2 == 0 else nc.scalar
                    eng.dma_start(xt[:], xT[dn * 128:(dn + 1) * 128,
                                            t0:t0 + 512])
                    xc.append(xt)
                pqk = [ps_qk.tile([128, 512], f32, tag="ps_qk",
                                  name=f"pqk{tq}_{i}") for i in range(4)]
                pv = [ps_v.tile([128, 256], f32, tag="ps_v",
                                name=f"pv{tq}_{i}") for i in range(4)]
                for dn in range(8):
                    for ch in range(4):
                        nc.tensor.matmul(
                            pqk[ch][:],
                            wqk_t[:, dn, ch * 128:(ch + 1) * 128],
                            xc[dn][:],
                            start=(dn == 0), stop=(dn == 7))
                    for tt in range(4):  # token tiles of 128 within quarter
                        nc.tensor.matmul(
                            pv[tt][:],
                            xc[dn][:, tt * 128:(tt + 1) * 128],
                            wv_t[:, dn, :],
                            start=(dn == 0), stop=(dn == 7))
                # psum -> sbuf copies (+ bias for q,k on ACT)
                for ch in range(4):
                    nc.scalar.activation(
                        chunks[ch][:, t0:t0 + 512], pqk[ch][:], IDENT,
                        bias=bqk_t[:, ch:ch + 1], scale=1.0)
                for tt in range(4):
                    tglob = tq * 4 + tt
                    dst = vhat[:, tglob * VW:(tglob + 1) * VW].rearrange(
                        "p (h c) -> p h c", h=HPC)[:, :, 0:HD]
                    nc.vector.tensor_copy(
                        dst, pv[tt][:].rearrange("p (h c) -> p h c", h=HPC))
                # rope + permute for this quarter, overlapped with the next
                # quarter's matmuls
                sl = slice(t0, t0 + 512)
                for (E, O, F, S) in ((qE, qO, qF, qS), (kE, kO, kF, kS)):
                    t1 = tmp_pool.tile([128, 512], bf16, tag="tmp")
                    t2 = tmp_pool.tile([128, 512], bf16, tag="tmp")
                    nc.vector.tensor_mul(t1[:], E[:, sl], cos_t[:, sl])
                    nc.vector.tensor_mul(t2[:], O[:, sl], sin_t[:, sl])
                    nc.vector.tensor_sub(F[:, sl], t1[:], t2[:])
                    t3 = tmp_pool.tile([128, 512], bf16, tag="tmp")
                    t4 = tmp_pool.tile([128, 512], bf16, tag="tmp")
                    nc.vector.tensor_mul(t3[:], E[:, sl], sin_t[:, sl])
                    nc.vector.tensor_mul(t4[:], O[:, sl], cos_t[:, sl])
                    nc.vector.tensor_add(S[:, sl], t3[:], t4[:])
                for (F, S, rA, rB) in ((qF, qS, qrA, qrB), (kF, kS, krA, krB)):
                    for hh in range(4):
                        dst = rA if hh < 2 else rB
                        r0 = (hh % 2) * 64
                        eng = nc.sync if hh % 2 == 0 else nc.scalar
                        eng.dma_start(dst[r0:r0 + 32, sl],
                                      F[hh * 32:(hh + 1) * 32, sl])
                        eng.dma_start(dst[r0 + 32:r0 + 64, sl],
                                      S[hh * 32:(hh + 1) * 32, sl])

        if dbg:
            nc.scalar.dma_start(dbg_qE, qE[:])
            nc.scalar.dma_start(dbg_qF, qF[:])
            nc.scalar.dma_start(dbg_vhat, vhat[:])

        # ---- phase 3: attention (head-outer for stationary reuse) ----
        # PV matmuls run one k-tile behind the S matmuls so the exp (ACT)
        # latency is hidden behind PE work and the PE never micro-stalls
        # (keeps the HAM clock gate at 8/8).
        otA = ot_pool.tile([128, T], bf16, tag="ot")
        otB = ot_pool.tile([128, T], bf16, tag="ot")
        with tc.tile_pool(name="ps_s", bufs=4, space="PSUM") as ps_s, \
             tc.tile_pool(name="ps_o", bufs=4, space="PSUM") as ps_o:
            for h in range(HPC):
                qr = qrA if h < 2 else qrB
                kr = krA if h < 2 else krB
                r0 = (h % 2) * 64
                po = [ps_o.tile([65, 512], f32, tag="ps_o",
                                name=f"po{h}_{i}") for i in range(NQC)]
                dens = [small.tile([1, 512], f32, tag="den",
                                   name=f"den{h}_{i}") for i in range(NQC)]

                def issue_s(kt):
                    k0 = kt * 128
                    qc_lo = kt // 4
                    ats = []
                    for qc in range(qc_lo, NQC):
                        q0 = qc * 512
                        col_lo = k0 - q0 if k0 > q0 else 0  # diag sub-range
                        pss = ps_s.tile([128, 512], f32, tag="ps_s",
                                        name=f"pss{h}_{kt}_{qc}")
                        nc.tensor.matmul(
                            pss[:, col_lo:512],
                            kr[r0:r0 + 64, k0:k0 + 128],
                            qr[r0:r0 + 64, q0 + col_lo:q0 + 512],
                            start=True, stop=True)
                        at = at_pool.tile([128, 512], bf16, tag="at",
                                          name=f"at{h}_{kt}_{qc}")
                        nc.scalar.activation(at[:, col_lo:512],
                                             pss[:, col_lo:512], EXP)
                        if qc == qc_lo and k0 >= q0:
                            nc.vector.tensor_mul(
                                at[:, col_lo:col_lo + 128],
                                at[:, col_lo:col_lo + 128], tri_t[:])
                        ats.append((qc, at, col_lo))
                        if dbg and h == 0 and kt == 0 and qc == 0:
                            nc.scalar.dma_start(dbg_at, at[:])
                    return ats

                def issue_pv(kt, ats):
                    for qc, at, col_lo in ats:
                        nc.tensor.matmul(
                            po[qc][:, col_lo:512],
                            vhat[:, kt * VW + h * 65:kt * VW + (h + 1) * 65],
                            at[:, col_lo:512],
                            start=(kt == 0), stop=(kt == 4 * qc + 3),
                            skip_group_check=True)
                        if kt == 4 * qc + 3:
                            # denominator row final: gather to a base-0 SBUF
                            # tile while the kt loop streams on
                            nc.vector.tensor_copy(dens[qc][:],
                                                  po[qc][64:65, :])

                prev = None
                for kt in range(NKT):
                    ats = issue_s(kt)
                    if prev is not None:
                        issue_pv(kt - 1, prev)
                    prev = ats
                issue_pv(NKT - 1, prev)

                if dbg and h == 0:
                    po_dump = rb_pool.tile([65, 512], f32, tag="podump")
                    nc.scalar.copy(po_dump[:], po[0][:])
                    nc.scalar.dma_start(dbg_po, po_dump[:])
                # normalize off the PE/ACT path: fast approx-recip per chunk,
                # GpSimd partition-broadcast, DVE multiply out of PSUM
                c = RECIP_APPROX_FAST_CONSTS
                for qc in range(NQC):
                    q0 = qc * 512
                    recip = small.tile([1, 512], f32, tag="recip",
                                       name=f"recip{h}_{qc}")
                    nc.vector._custom_dve(
                        RECIPROCAL_APPROX_FAST, out=recip[:],
                        in0=dens[qc][:], s0=c["s0"], s1=c["s1"],
                        imm2=c["imm2"])
                    rb = rb_pool.tile([64, 512], f32, tag="rb",
                                      name=f"rb{h}_{qc}")
                    nc.gpsimd.partition_broadcast(rb[:], recip[:])
                    ot = otA if h < 2 else otB
                    nc.vector.tensor_mul(ot[r0:r0 + 64, q0:q0 + 512],
                                         po[qc][0:64, :], rb[:])

            if dbg:
                nc.scalar.dma_start(dbg_qrA, qrA[:])
                nc.scalar.dma_start(dbg_ot, otA[:])

            # ---- phase 4: projection (stationary reuse across qc) ----
            for oc in range(8):
                pj = [ps_s.tile([128, 512], f32, tag="ps_s",
                                name=f"pj{oc}_{i}") for i in range(NQC)]
                for hd in range(2):
                    src = otA if hd == 0 else otB
                    for qc in range(NQC):
                        nc.tensor.matmul(
                            pj[qc][:], wp_t[:, hd, oc * 128:(oc + 1) * 128],
                            src[:, qc * 512:qc * 512 + 512],
                            start=(hd == 0), stop=(hd == 1))
                for qc in range(NQC):
                    ob = po_pool.tile([128, 512], bf16, tag="po",
                                      name=f"ob{oc}_{qc}")
                    ceng = nc.vector if qc % 2 == 0 else nc.scalar
                    if qc % 2 == 0:
                        ceng.tensor_copy(ob[:], pj[qc][:])
                    else:
                        ceng.copy(ob[:], pj[qc][:])
                    deng = nc.sync if qc % 2 == 0 else nc.scalar
                    deng.dma_start(
                        outT[oc * 128:(oc + 1) * 128,
                             qc * 512:qc * 512 + 512], ob[:])

    nc.compile()
    return nc


_NC = None


def _get_nc():
    global _NC
    if _NC is None:
        _NC = _build()
    return _NC


def _sel4():
    """[128, 256] one-hot stationary: column qc*64+p reads partition 32*qc."""
    s = np.zeros((128, 256), dtype=np.float32)
    for qc in range(4):
        s[32 * qc, qc * 64:(qc + 1) * 64] = 1.0
    return s


def _host_prep(x, Wqkv, bqkv, Wproj, bproj, pos):
    """Build the 8 per-core input maps."""
    import ml_dtypes
    nbf16 = ml_dtypes.bfloat16

    x = np.asarray(x, dtype=np.float32)
    Wqkv = np.asarray(Wqkv, dtype=np.float32)
    bqkv = np.asarray(bqkv, dtype=np.float32)
    Wproj = np.asarray(Wproj, dtype=np.float32)
    bproj = np.asarray(bproj, dtype=np.float32)
    pos = int(np.asarray(pos))

    scale = HD ** -0.5
    # rope tables, layout [128 = 4 heads x 32 thetas (h-major), T]
    theta = 1.0 / BASE ** (np.arange(HALF, dtype=np.float32) / HALF)
    angles = np.outer(np.arange(pos, pos + T, dtype=np.float32), theta)  # [T,32]
    cosT = np.cos(angles).T.astype(np.float32)  # [32, T]
    sinT = np.sin(angles).T.astype(np.float32)
    cos4 = np.ascontiguousarray(np.tile(cosT, (4, 1))).astype(nbf16)  # [128, T]
    sin4 = np.ascontiguousarray(np.tile(sinT, (4, 1))).astype(nbf16)

    tri = np.tril(np.ones((128, 128), dtype=np.float32)).T  # m[p,j]=1 if p<=j
    tri = np.ascontiguousarray(tri).astype(nbf16)

    in_maps = []
    for c in range(8):
        b, hg = c // 4, c % 4
        heads = [4 * hg + h for h in range(HPC)]
        permE = np.array([h * HD + 2 * i for h in heads for i in range(HALF)])
        permO = permE + 1
        wqk_np = np.concatenate([
            Wqkv[:, permE] * scale,          # qE
            Wqkv[:, permO] * scale,          # qO
            Wqkv[:, D + permE],              # kE
            Wqkv[:, D + permO],              # kO
        ], axis=1)
        bqk_np = np.stack([
            bqkv[permE] * scale, bqkv[permO] * scale,
            bqkv[D + permE], bqkv[D + permO],
        ], axis=1)
        wv_np = Wqkv[:, 2 * D + 256 * hg: 2 * D + 256 * (hg + 1)]
        wp_np = Wproj[256 * hg: 256 * (hg + 1), :]
        in_maps.append({
            "xT": np.ascontiguousarray(x[b].T).astype(nbf16),
            "wqk": np.ascontiguousarray(wqk_np).astype(nbf16),
            "wv": np.ascontiguousarray(wv_np).astype(nbf16),
            "wp": np.ascontiguousarray(wp_np).astype(nbf16),
            "bqk": np.ascontiguousarray(bqk_np, dtype=np.float32),
            "cos4": cos4,
            "sin4": sin4,
            "trimask": tri,
            "ones64_d": np.ones((1, 64), dtype=np.float32),

            "ones_pat": np.ones((128, 64), dtype=nbf16),
        })
    const_vec = bqkv[2 * D:] @ Wproj + bproj  # exact host-side bias handling
    return in_maps, const_vec


def kernel(x, Wqkv, bqkv, Wproj, bproj, pos, **kw):
    in_maps, const_vec = _host_prep(x, Wqkv, bqkv, Wproj, bproj, pos)
    nc = _get_nc()
    res = run_bass_kernel_spmd(nc, in_maps, core_ids=list(range(8))).results
    out = np.empty((B, T, D), dtype=np.float32)
    for b in range(B):
        acc = res[4 * b]["outT"].astype(np.float32)
        for c in range(4 * b + 1, 4 * b + 4):
            acc += res[c]["outT"].astype(np.float32)
        out[b] = acc.T + const_vec
    return out


# revision 45
# speedup vs baseline: 1.7484x; 1.2568x over previous
"""Multi-head self-attention (RoPE, causal) Trainium2 Bass kernel.

Full inputs in, full output out. Sharding: 8 cores = 2 batch x 4 head-groups
(4 heads each). Per core: qkv projection, RoPE on DVE, streaming causal
attention (S^T orientation: softmax reduction along partitions via a
ones-column in V-hat), output projection partial. Host sums the 4 per-batch
partials and adds the (bv @ Wproj + bproj) constant.

v2: all matmul operands in bf16 (PSUM accumulation stays fp32) — lower PE
power (avoids the sustained K=4/8 clock throttle seen with f32r), fast
weight loads, half the DMA bytes. Reciprocal via the fast custom-DVE
approximation. RoPE in bf16 for DVE 2x throughput. Output staged bf16.

Self-contained: hardcodes all shapes for B=2, T=2048, D=1024, H=16, hd=64.
"""
from contextlib import ExitStack

import numpy as np

from concourse import bacc, mybir, tile
from concourse.bass_utils import run_bass_kernel_spmd
from concourse.dve_ops import RECIP_APPROX_FAST_CONSTS, RECIPROCAL_APPROX_FAST

f32 = mybir.dt.float32
f32r = mybir.dt.float32r
bf16 = mybir.dt.bfloat16
EXP = mybir.ActivationFunctionType.Exp
IDENT = mybir.ActivationFunctionType.Identity

B, T, D = 2, 2048, 1024
H, HD = 16, 64
HALF = HD // 2  # 32
HPC = 4  # heads per core
BASE = 10000.0
NTQ = 4  # token quarters of 512 (qkv phase)
NQC = 4  # query chunks of 512 (attention phase)
NKT = 16  # key tiles of 128
VW = HPC * (HD + 1)  # 260: v-hat columns per token tile


def _build(dbg=False):
    nc = bacc.Bacc("TRN2", target_bir_lowering=False, debug=False, num_devices=8)

    xT = nc.dram_tensor("xT", [D, T], bf16, kind="ExternalInput").ap()
    wqk = nc.dram_tensor("wqk", [D, 512], bf16, kind="ExternalInput").ap()
    wv = nc.dram_tensor("wv", [D, 256], bf16, kind="ExternalInput").ap()
    wp = nc.dram_tensor("wp", [256, D], bf16, kind="ExternalInput").ap()
    bqk = nc.dram_tensor("bqk", [128, 4], f32, kind="ExternalInput").ap()
    cos4 = nc.dram_tensor("cos4", [128, T], bf16, kind="ExternalInput").ap()
    sin4 = nc.dram_tensor("sin4", [128, T], bf16, kind="ExternalInput").ap()
    trimask = nc.dram_tensor("trimask", [128, 128], bf16, kind="ExternalInput").ap()
    ones64_d = nc.dram_tensor("ones64_d", [1, 64], f32r, kind="ExternalInput").ap()
    ones_pat = nc.dram_tensor("ones_pat", [128, 64], bf16, kind="ExternalInput").ap()
    outT = nc.dram_tensor("outT", [D, T], bf16, kind="ExternalOutput").ap()
    if dbg:
        dbg_qE = nc.dram_tensor("dbg_qE", [128, T], bf16, kind="ExternalOutput").ap()
        dbg_qF = nc.dram_tensor("dbg_qF", [128, T], bf16, kind="ExternalOutput").ap()
        dbg_qrA = nc.dram_tensor("dbg_qrA", [128, T], bf16, kind="ExternalOutput").ap()
        dbg_vhat = nc.dram_tensor("dbg_vhat", [128, NKT * VW], bf16, kind="ExternalOutput").ap()
        dbg_at = nc.dram_tensor("dbg_at", [128, 512], bf16, kind="ExternalOutput").ap()
        dbg_po = nc.dram_tensor("dbg_po", [65, 512], mybir.dt.float32, kind="ExternalOutput").ap()
        dbg_ot = nc.dram_tensor("dbg_ot", [128, T], bf16, kind="ExternalOutput").ap()

    with tile.TileContext(nc) as tc, ExitStack() as ctx:
        consts = ctx.enter_context(tc.tile_pool(name="consts", bufs=1))
        wpool = ctx.enter_context(tc.tile_pool(name="wpool", bufs=1))
        xt_pool = ctx.enter_context(tc.tile_pool(name="xt", bufs=10))
        qkstage = ctx.enter_context(tc.tile_pool(name="qkstage", bufs=14))
        tmp_pool = ctx.enter_context(tc.tile_pool(name="tmp", bufs=2))
        vh_pool = ctx.enter_context(tc.tile_pool(name="vh", bufs=1))
        at_pool = ctx.enter_context(tc.tile_pool(name="at", bufs=10))
        small = ctx.enter_context(tc.tile_pool(name="small", bufs=8))
        rb_pool = ctx.enter_context(tc.tile_pool(name="rb", bufs=2))
        ot_pool = ctx.enter_context(tc.tile_pool(name="ot", bufs=2))
        po_pool = ctx.enter_context(tc.tile_pool(name="po", bufs=6))

        # ---- weights on the sync queue interleaved with first x chunks;
        # ---- everything not needed immediately on the scalar HWDGE queue.
        wqk_t = wpool.tile([128, 8, 512], bf16, tag="wqk_t")
        wv_t = wpool.tile([128, 8, 256], bf16, tag="wv_t")
        wp_t = wpool.tile([128, 2, D], bf16, tag="wp_t")
        cos_t = consts.tile([128, T], bf16, tag="cos_t")
        sin_t = consts.tile([128, T], bf16, tag="sin_t")
        tri_t = consts.tile([128, 128], bf16, tag="tri_t")
        bqk_t = consts.tile([128, 4], f32, tag="bqk_t")
        ones64 = consts.tile([1, 64], f32r, tag="ones64")


        for dn in range(8):
            nc.scalar.dma_start(wv_t[:, dn, :], wv[dn * 128:(dn + 1) * 128, :])
        nc.scalar.dma_start(bqk_t[:], bqk)
        nc.scalar.dma_start(cos_t[:], cos4)
        nc.scalar.dma_start(sin_t[:], sin4)
        nc.scalar.dma_start(tri_t[:], trimask)
        nc.scalar.dma_start(ones64[:], ones64_d)
        for hd in range(2):
            nc.scalar.dma_start(wp_t[:, hd, :], wp[hd * 128:(hd + 1) * 128, :])

        # v-hat: [128, 16 tok-tiles x (4 heads x 65)]; col 64 of each head = 1.0
        vhat = vh_pool.tile([128, NKT * VW], bf16, tag="vhat")
        vh_ones = vhat[:, :].rearrange("p (t h c) -> p t h c", t=NKT,
                                       h=HPC)[:, :, :, HD:HD + 1]
        nc.scalar.dma_start(vh_ones,
                            ones_pat.rearrange("p (t h) -> p t h", t=NKT)[:, :, :, None])

        # qkv^T output stage tensors [128, T] each
        qE = qkstage.tile([128, T], bf16, tag="qks")
        qO = qkstage.tile([128, T], bf16, tag="qks")
        kE = qkstage.tile([128, T], bf16, tag="qks")
        kO = qkstage.tile([128, T], bf16, tag="qks")
        chunks = [qE, qO, kE, kO]
        qF = qkstage.tile([128, T], bf16, tag="qks")
        qS = qkstage.tile([128, T], bf16, tag="qks")
        kF = qkstage.tile([128, T], bf16, tag="qks")
        kS = qkstage.tile([128, T], bf16, tag="qks")
        qrA = qkstage.tile([128, T], bf16, tag="qks")
        qrB = qkstage.tile([128, T], bf16, tag="qks")
        # per-head K stationaries, zero-padded to the full 128 partitions:
        # a 64-partition stationary never registers as "busy" to the HAM
        # clock gate and pins the PE at half clock
        kr_pad = [qkstage.tile([128, T], bf16, tag="qks",
                               name=f"krp{hh}") for hh in range(4)]
        for hh in range(4):
            r0 = (hh % 2) * 64
            nc.gpsimd.memset(kr_pad[hh][64 - r0:128 - r0, :], 0.0)

        # ---- phase 1: qkv matmuls ----
        with tc.tile_pool(name="ps_qk", bufs=4, space="PSUM") as ps_qk, \
             tc.tile_pool(name="ps_v", bufs=4, space="PSUM") as ps_v:
            for tq in range(NTQ):
                t0 = tq * 512
                xc = []
                for dn in range(8):
                    xt = xt_pool.tile([128, 512], bf16, tag="xt")
                    if tq == 0:
                        # interleave weight + activation loads so the first
                        # matmul can start after two small DMAs
                        nc.sync.dma_start(wqk_t[:, dn, :],
                                          wqk[dn * 128:(dn + 1) * 128, :])
                    # alternate x chunks between both HWDGE queues
                    eng = nc.sync if dn % 2 == 0 else nc.scalar
                    eng.dma_start(xt[:], xT[dn * 128:(dn + 1) * 128,
                                            t0:t0 + 512])
                    xc.append(xt)
                pqk = [ps_qk.tile([128, 512], f32, tag="ps_qk",
                                  name=f"pqk{tq}_{i}") for i in range(4)]
                pv = [ps_v.tile([128, 256], f32, tag="ps_v",
                                name=f"pv{tq}_{i}") for i in range(4)]
                for dn in range(8):
                    for ch in range(4):
                        nc.tensor.matmul(
                            pqk[ch][:],
                            wqk_t[:, dn, ch * 128:(ch + 1) * 128],
                            xc[dn][:],
                            start=(dn == 0), stop=(dn == 7))
                    for tt in range(4):  # token tiles of 128 within quarter
                        nc.tensor.matmul(
                            pv[tt][:],
                            xc[dn][:, tt * 128:(tt + 1) * 128],
                            wv_t[:, dn, :],
                            start=(dn == 0), stop=(dn == 7))
                # psum -> sbuf copies (+ bias for q,k on ACT)
                for ch in range(4):
                    nc.scalar.activation(
                        chunks[ch][:, t0:t0 + 512], pqk[ch][:], IDENT,
                        bias=bqk_t[:, ch:ch + 1], scale=1.0)
                for tt in range(4):
                    tglob = tq * 4 + tt
                    dst = vhat[:, tglob * VW:(tglob + 1) * VW].rearrange(
                        "p (h c) -> p h c", h=HPC)[:, :, 0:HD]
                    nc.vector.tensor_copy(
                        dst, pv[tt][:].rearrange("p (h c) -> p h c", h=HPC))
                # rope + permute for this quarter, overlapped with the next
                # quarter's matmuls
                sl = slice(t0, t0 + 512)
                for (E, O, F, S) in ((qE, qO, qF, qS), (kE, kO, kF, kS)):
                    t1 = tmp_pool.tile([128, 512], bf16, tag="tmp")
                    t2 = tmp_pool.tile([128, 512], bf16, tag="tmp")
                    nc.vector.tensor_mul(t1[:], E[:, sl], cos_t[:, sl])
                    nc.vector.tensor_mul(t2[:], O[:, sl], sin_t[:, sl])
                    nc.vector.tensor_sub(F[:, sl], t1[:], t2[:])
                    t3 = tmp_pool.tile([128, 512], bf16, tag="tmp")
                    t4 = tmp_pool.tile([128, 512], bf16, tag="tmp")
                    nc.vector.tensor_mul(t3[:], E[:, sl], sin_t[:, sl])
                    nc.vector.tensor_mul(t4[:], O[:, sl], cos_t[:, sl])
                    nc.vector.tensor_add(S[:, sl], t3[:], t4[:])
                for hh in range(4):
                    dst = qrA if hh < 2 else qrB
                    r0 = (hh % 2) * 64
                    eng = nc.sync if hh % 2 == 0 else nc.scalar
                    eng.dma_start(dst[r0:r0 + 32, sl],
                                  qF[hh * 32:(hh + 1) * 32, sl])
                    eng.dma_start(dst[r0 + 32:r0 + 64, sl],
                                  qS[hh * 32:(hh + 1) * 32, sl])
                    eng.dma_start(kr_pad[hh][r0:r0 + 32, sl],
                                  kF[hh * 32:(hh + 1) * 32, sl])
                    eng.dma_start(kr_pad[hh][r0 + 32:r0 + 64, sl],
                                  kS[hh * 32:(hh + 1) * 32, sl])

        if dbg:
            nc.scalar.dma_start(dbg_qE, qE[:])
            nc.scalar.dma_start(dbg_qF, qF[:])
            nc.scalar.dma_start(dbg_vhat, vhat[:])

        # ---- phase 3: attention (head-outer for stationary reuse) ----
        # PV matmuls run one k-tile behind the S matmuls so the exp (ACT)
        # latency is hidden behind PE work and the PE never micro-stalls
        # (keeps the HAM clock gate at 8/8).
        otA = ot_pool.tile([128, T], bf16, tag="ot")
        otB = ot_pool.tile([128, T], bf16, tag="ot")
        with tc.tile_pool(name="ps_s", bufs=4, space="PSUM") as ps_s, \
             tc.tile_pool(name="ps_o", bufs=4, space="PSUM") as ps_o:
            for h in range(HPC):
                qr = qrA if h < 2 else qrB
                kr = kr_pad[h]
                r0 = (h % 2) * 64
                po = [ps_o.tile([65, 512], f32, tag="ps_o",
                                name=f"po{h}_{i}") for i in range(NQC)]
                dens = [small.tile([1, 512], f32, tag="den",
                                   name=f"den{h}_{i}") for i in range(NQC)]

                def issue_s(kt):
                    k0 = kt * 128
                    qc_lo = kt // 4
                    ats = []
                    for qc in range(qc_lo, NQC):
                        q0 = qc * 512
                        col_lo = k0 - q0 if k0 > q0 else 0  # diag sub-range
                        pss = ps_s.tile([128, 512], f32, tag="ps_s",
                                        name=f"pss{h}_{kt}_{qc}")
                        nc.tensor.matmul(
                            pss[:, col_lo:512],
                            kr[:, k0:k0 + 128],
                            qr[:, q0 + col_lo:q0 + 512],
                            start=True, stop=True)
                        at = at_pool.tile([128, 512], bf16, tag="at",
                                          name=f"at{h}_{kt}_{qc}")
                        nc.scalar.activation(at[:, col_lo:512],
                                             pss[:, col_lo:512], EXP)
                        if qc == qc_lo and k0 >= q0:
                            nc.vector.tensor_mul(
                                at[:, col_lo:col_lo + 128],
                                at[:, col_lo:col_lo + 128], tri_t[:])
                        ats.append((qc, at, col_lo))
                        if dbg and h == 0 and kt == 0 and qc == 0:
                            nc.scalar.dma_start(dbg_at, at[:])
                    return ats

                def issue_pv(kt, ats):
                    for qc, at, col_lo in ats:
                        nc.tensor.matmul(
                            po[qc][:, col_lo:512],
                            vhat[:, kt * VW + h * 65:kt * VW + (h + 1) * 65],
                            at[:, col_lo:512],
                            start=(kt == 0), stop=(kt == 4 * qc + 3),
                            skip_group_check=True)
                        if kt == 4 * qc + 3:
                            # denominator row final: gather to a base-0 SBUF
                            # tile while the kt loop streams on
                            nc.vector.tensor_copy(dens[qc][:],
                                                  po[qc][64:65, :])

                prev = None
                for kt in range(NKT):
                    ats = issue_s(kt)
                    if prev is not None:
                        issue_pv(kt - 1, prev)
                    prev = ats
                issue_pv(NKT - 1, prev)

                if dbg and h == 0:
                    po_dump = rb_pool.tile([65, 512], f32, tag="podump")
                    nc.scalar.copy(po_dump[:], po[0][:])
                    nc.scalar.dma_start(dbg_po, po_dump[:])
                # normalize off the PE/ACT path: fast approx-recip per chunk,
                # GpSimd partition-broadcast, DVE multiply out of PSUM
                c = RECIP_APPROX_FAST_CONSTS
                for qc in range(NQC):
                    q0 = qc * 512
                    recip = small.tile([1, 512], f32, tag="recip",
                                       name=f"recip{h}_{qc}")
                    nc.vector._custom_dve(
                        RECIPROCAL_APPROX_FAST, out=recip[:],
                        in0=dens[qc][:], s0=c["s0"], s1=c["s1"],
                        imm2=c["imm2"])
                    rb = rb_pool.tile([64, 512], f32, tag="rb",
                                      name=f"rb{h}_{qc}")
                    nc.gpsimd.partition_broadcast(rb[:], recip[:])
                    ot = otA if h < 2 else otB
                    nc.vector.tensor_mul(ot[r0:r0 + 64, q0:q0 + 512],
                                         po[qc][0:64, :], rb[:])

            if dbg:
                nc.scalar.dma_start(dbg_qrA, qrA[:])
                nc.scalar.dma_start(dbg_ot, otA[:])

            # ---- phase 4: projection (stationary reuse across qc) ----
            for oc in range(8):
                pj = [ps_s.tile([128, 512], f32, tag="ps_s",
                                name=f"pj{oc}_{i}") for i in range(NQC)]
                for hd in range(2):
                    src = otA if hd == 0 else otB
                    for qc in range(NQC):
                        nc.tensor.matmul(
                            pj[qc][:], wp_t[:, hd, oc * 128:(oc + 1) * 128],
                            src[:, qc * 512:qc * 512 + 512],
                            start=(hd == 0), stop=(hd == 1))
                for qc in range(NQC):
                    ob = po_pool.tile([128, 512], bf16, tag="po",
                                      name=f"ob{oc}_{qc}")
                    ceng = nc.vector if qc % 2 == 0 else nc.scalar
                    if qc % 2 == 0:
                        ceng.tensor_copy(ob[:], pj[qc][:])
                    else:
                        ceng.copy(ob[:], pj[qc][:])
                    deng = nc.sync if qc % 2 == 0 else nc.scalar
                    deng.dma_start(
                        outT[oc * 128:(oc + 1) * 128,
                             qc * 512:qc * 512 + 512], ob[:])

    nc.compile()
    return nc


_NC = None


def _get_nc():
    global _NC
    if _NC is None:
        _NC = _build()
    return _NC


def _sel4():
    """[128, 256] one-hot stationary: column qc*64+p reads partition 32*qc."""
    s = np.zeros((128, 256), dtype=np.float32)
    for qc in range(4):
        s[32 * qc, qc * 64:(qc + 1) * 64] = 1.0
    return s


def _host_prep(x, Wqkv, bqkv, Wproj, bproj, pos):
    """Build the 8 per-core input maps."""
    import ml_dtypes
    nbf16 = ml_dtypes.bfloat16

    x = np.asarray(x, dtype=np.float32)
    Wqkv = np.asarray(Wqkv, dtype=np.float32)
    bqkv = np.asarray(bqkv, dtype=np.float32)
    Wproj = np.asarray(Wproj, dtype=np.float32)
    bproj = np.asarray(bproj, dtype=np.float32)
    pos = int(np.asarray(pos))

    scale = HD ** -0.5
    # rope tables, layout [128 = 4 heads x 32 thetas (h-major), T]
    theta = 1.0 / BASE ** (np.arange(HALF, dtype=np.float32) / HALF)
    angles = np.outer(np.arange(pos, pos + T, dtype=np.float32), theta)  # [T,32]
    cosT = np.cos(angles).T.astype(np.float32)  # [32, T]
    sinT = np.sin(angles).T.astype(np.float32)
    cos4 = np.ascontiguousarray(np.tile(cosT, (4, 1))).astype(nbf16)  # [128, T]
    sin4 = np.ascontiguousarray(np.tile(sinT, (4, 1))).astype(nbf16)

    tri = np.tril(np.ones((128, 128), dtype=np.float32)).T  # m[p,j]=1 if p<=j
    tri = np.ascontiguousarray(tri).astype(nbf16)

    in_maps = []
    for c in range(8):
        b, hg = c // 4, c % 4
        heads = [4 * hg + h for h in range(HPC)]
        permE = np.array([h * HD + 2 * i for h in heads for i in range(HALF)])
        permO = permE + 1
        wqk_np = np.concatenate([
            Wqkv[:, permE] * scale,          # qE
            Wqkv[:, permO] * scale,          # qO
            Wqkv[:, D + permE],              # kE
            Wqkv[:, D + permO],              # kO
        ], axis=1)
        bqk_np = np.stack([
            bqkv[permE] * scale, bqkv[permO] * scale,
            bqkv[D + permE], bqkv[D + permO],
        ], axis=1)
        wv_np = Wqkv[:, 2 * D + 256 * hg: 2 * D + 256 * (hg + 1)]
        wp_np = Wproj[256 * hg: 256 * (hg + 1), :]
        in_maps.append({
            "xT": np.ascontiguousarray(x[b].T).astype(nbf16),
            "wqk": np.ascontiguousarray(wqk_np).astype(nbf16),
            "wv": np.ascontiguousarray(wv_np).astype(nbf16),
            "wp": np.ascontiguousarray(wp_np).astype(nbf16),
            "bqk": np.ascontiguousarray(bqk_np, dtype=np.float32),
            "cos4": cos4,
            "sin4": sin4,
            "trimask": tri,
            "ones64_d": np.ones((1, 64), dtype=np.float32),

            "ones_pat": np.ones((128, 64), dtype=nbf16),
        })
    const_vec = bqkv[2 * D:] @ Wproj + bproj  # exact host-side bias handling
    return in_maps, const_vec


def kernel(x, Wqkv, bqkv, Wproj, bproj, pos, **kw):
    in_maps, const_vec = _host_prep(x, Wqkv, bqkv, Wproj, bproj, pos)
    nc = _get_nc()
    res = run_bass_kernel_spmd(nc, in_maps, core_ids=list(range(8))).results
    out = np.empty((B, T, D), dtype=np.float32)
    for b in range(B):
        acc = res[4 * b]["outT"].astype(np.float32)
        for c in range(4 * b + 1, 4 * b + 4):
            acc += res[c]["outT"].astype(np.float32)
        out[b] = acc.T + const_vec
    return out
